# revision 19
# baseline (speedup 1.0000x reference)
"""Trainium2 Bass kernel for nn_DiffTreeMachine (B=64, L=16, F=64, R=1023).

Data-parallel over batch: 8 NeuronCores x 8 batches each.

Per core, the computation is
  A_w[b]   = sum_l w_w[b,l] * x[b,l]          (4 weighted reductions, w in {car,cdr,cons1,cons2})
  car      = A_0 @ D_l^T
  cdr      = A_1 @ D_r^T
  cons     = A_2 @ E_l^T + A_3 @ E_r^T + outer(root_filler, root_role)
  ent_w[b] = -(sum_l p log(p+1e-12)) / log(L),  max_w[b] = max_l p

Device mapping:
  Stage 1: for each f (64) and r-chunk (8x128): matmul with x tile as the
    *stationary* operand and a sparse (128, 32) weight-block as the moving
    operand: out[(r),(w,b)] = x_slice.T @ wblk.  This lands A^T directly with
    r on partitions (the layout stage 2 needs for its contraction) with no
    separate transpose pass.  fp32 exact.
  Stage 2: out[(f,b2),(t)] = sum_rc Astage[rc].T @ W^T[rc]  accumulated in
    PSUM over 8 r-chunks, in float32r (PE full rate, ~11-bit-mantissa
    operand rounding, rel err ~1.5e-4).  The cons output accumulates two
    chains (E_l, E_r) plus a K=1 rank-1 matmul for the root term in one
    PSUM group.
"""

import math
import os
import sys
import threading

import numpy as np

for _p in ("/opt/trn_rl_repo", "/root/.axon_site/_ro/trn_rl_repo"):
    if os.path.isdir(_p) and _p not in sys.path:
        sys.path.insert(0, _p)

B, L, F, R = 64, 16, 64, 1023
NCORES = 8
BLOC = B // NCORES          # 8 batches per core
RP = 1024                   # R padded
NRC = 8                     # r-chunks of 128 (last has 127 real rows)
W4 = 4                      # four weight sets
ACOLS = W4 * BLOC * F       # astage cols: w*512 + b*64 + f

_runner_lock = threading.Lock()
_runner = None


def _build_program():
    import concourse.tile as tile
    import concourse.mybir as mybir
    from concourse import bacc

    f32 = mybir.dt.float32
    f32r = mybir.dt.float32r

    nc = bacc.Bacc("TRN2", target_bir_lowering=False, debug=False,
                   num_devices=NCORES)

    xs = nc.declare_dram_parameter("xs", [BLOC * L, F * R], f32, isOutput=False)
    wblk = nc.declare_dram_parameter("wblk", [BLOC * L, 32], f32, isOutput=False)
    # W^T padded on both dims: rows r->1024 (zero), cols t->1024 (zero)
    wts = nc.declare_dram_parameter("wts", [W4, RP, RP], f32r, isOutput=False)
    rfrr = nc.declare_dram_parameter("rfrr", [1, 4 * 128 + RP], f32r, isOutput=False)
    pw = nc.declare_dram_parameter("pw", [32, L], f32, isOutput=False)

# row layout: partition p = f*2 + b2 (b2 = batch index within the pair)
    out_car = nc.declare_dram_parameter("out_car", [4, F * 2, R], f32, isOutput=True)
    out_cdr = nc.declare_dram_parameter("out_cdr", [4, F * 2, R], f32, isOutput=True)
    out_cons = nc.declare_dram_parameter("out_cons", [4, F * 2, R], f32, isOutput=True)
    out_stats = nc.declare_dram_parameter("out_stats", [32, 2], f32, isOutput=True)

    outs = {"car": out_car, "cdr": out_cdr, "cons": out_cons}

    with tile.TileContext(nc) as tc:
        with (
            tc.tile_pool(name="const", bufs=1) as const,
            tc.tile_pool(name="astage", bufs=1) as apool,
            tc.tile_pool(name="xp", bufs=10) as xpool,
            tc.tile_pool(name="wtp", bufs=2) as wtpool,
            tc.tile_pool(name="obp", bufs=3) as obpool,
            tc.tile_pool(name="ps1", bufs=1, space="PSUM") as ps1,
            tc.tile_pool(name="ps2", bufs=3, space="PSUM") as ps2,
        ):
            wblk_sb = const.tile([BLOC * L, 32], f32)
            nc.sync.dma_start(wblk_sb[:], wblk[:])
            rfrr_sb = const.tile([1, 4 * 128 + RP], f32r)
            nc.sync.dma_start(rfrr_sb[:], rfrr[:])
            pw_sb = const.tile([32, L], f32)
            nc.sync.dma_start(pw_sb[:], pw[:])

            # ---- stats: entropy + max of the four weight matrices ----
            eps = const.tile([32, 1], f32)
            nc.vector.memset(eps[:], 1e-12)
            lnp = const.tile([32, L], f32)
            nc.scalar.activation(lnp[:], pw_sb[:],
                                 mybir.ActivationFunctionType.Ln, bias=eps[:])
            plnp = const.tile([32, L], f32)
            nc.vector.tensor_mul(plnp[:], pw_sb[:], lnp[:])
            st = const.tile([32, 2], f32)
            nc.vector.reduce_sum(st[:, 0:1], plnp[:], axis=mybir.AxisListType.X)
            nc.vector.tensor_scalar_mul(st[:, 0:1], st[:, 0:1], -1.0 / math.log(L))
            nc.vector.reduce_max(st[:, 1:2], pw_sb[:], axis=mybir.AxisListType.X)
            nc.sync.dma_start(out_stats[:], st[:])

            # ---- A^T staging: 8 r-chunk tiles, cols = w*512 + b*64 + f ----
            astage = []
            for rc in range(NRC):
                a_t = apool.tile([128, ACOLS], f32r, tag=f"a{rc}", name=f"astage{rc}")
                astage.append(a_t)
            # r = 1023 row (last row of last chunk) is never written by
            # stage 1; zero it so stage 2 reads 0 * 0 there.  DVE partition
            # base must be 32-aligned, so clear the whole last group (stage-1
            # copies then overwrite rows 96..126).
            nc.vector.memset(astage[NRC - 1][96:128, :].bitcast(f32), 0.0)

            # ---- stage 1 ----
            # loop f-blocks of 8; within a block do r-chunk halves of 4 so
            # only 4 PSUM banks are needed.
            for fb in range(8):
                x_ts = []
                for f8 in range(8):
                    f = fb * 8 + f8
                    x_t = xpool.tile([128, R], f32, tag="x", name=f"x{f}")
                    nc.sync.dma_start(x_t[:], xs[:, f * R:(f + 1) * R])
                    x_ts.append(x_t)
                for rch in range(2):
                    pst = []
                    for rc2 in range(4):
                        p_t = ps1.tile([128, 256], f32, tag=f"p{rc2}",
                                       name=f"ps1_{rc2}")
                        pst.append(p_t)
                    for f8 in range(8):
                        for rc2 in range(4):
                            rc = rch * 4 + rc2
                            nr = 128 if rc < 7 else 127
                            nc.tensor.matmul(
                                pst[rc2][0:nr, f8 * 32:(f8 + 1) * 32],
                                x_ts[f8][:, rc * 128: rc * 128 + nr],
                                wblk_sb[:],
                                start=True, stop=True,
                            )
                    for rc2 in range(4):
                        rc = rch * 4 + rc2
                        nr = 128 if rc < 7 else 127
                        # psum cols are f8*32 + w*8 + b; astage cols are
                        # w*512 + b*64 + f.  One strided copy per w.
                        src = pst[rc2].rearrange("p (f8 w b) -> p w b f8",
                                                 w=W4, b=BLOC)
                        dst = astage[rc].rearrange("p (w b f) -> p w b f",
                                                   b=BLOC, f=F)
                        for w in range(W4):
                            nc.vector.tensor_copy(
                                dst[0:nr, w, :, fb * 8:(fb + 1) * 8],
                                src[0:nr, w, :, :],
                            )

            # ---- stage 2 ----
            # section -> list of (w_idx, wt_idx, tag_group)
            sections = [
                ("car", [(0, 0, "A")]),
                ("cdr", [(1, 1, "B")]),
                ("cons", [(2, 2, "A"), (3, 3, "B")]),
            ]
            for sec, chains in sections:
                od = outs[sec]
                for th in range(2):
                    t0 = th * 512
                    nt = 512 if th == 0 else R - 512  # real (unpadded) cols
                    wt_sb = {}
                    for (w, wi, tg) in chains:
                        for rc in range(NRC):
                            wt_t = wtpool.tile([128, 512], f32r,
                                               tag=f"{tg}{rc}",
                                               name=f"wt_{sec}_{th}_{w}_{rc}")
                            nc.sync.dma_start(
                                wt_t[:],
                                wts[wi, rc * 128:(rc + 1) * 128, t0:t0 + 512])
                            wt_sb[(w, rc)] = wt_t
                    n_mm = len(chains) * NRC + (1 if sec == "cons" else 0)
                    for bp in range(4):
                        acc = ps2.tile([128, 512], f32, tag="acc",
                                       name=f"acc_{sec}_{th}_{bp}")
                        mm = 0
                        for (w, wi, tg) in chains:
                            for rc in range(NRC):
                                lhsT = astage[rc][:, w * 512 + bp * 128:
                                                  w * 512 + (bp + 1) * 128]
                                nc.tensor.matmul(
                                    acc[:],
                                    lhsT,
                                    wt_sb[(w, rc)][:],
                                    start=(mm == 0), stop=(mm == n_mm - 1),
                                )
                                mm += 1
                        if sec == "cons":
                            nc.tensor.matmul(
                                acc[:],
                                rfrr_sb[0:1, bp * 128:(bp + 1) * 128],
                                rfrr_sb[0:1, 512 + t0: 512 + t0 + 512],
                                start=False, stop=True,
                            )
                        ob = obpool.tile([128, 512], f32, tag="ob",
                                         name=f"ob_{sec}_{th}_{bp}")
                        nc.vector.tensor_copy(ob[:], acc[:])
                        nc.sync.dma_start(od[bp, :, t0:t0 + nt], ob[:, 0:nt])

    nc.compile()
    return nc


def _prep_inputs(x, car_w, cdr_w, cons1_w, cons2_w, root_filler,
                 D_l, D_r, E_l, E_r, root_role):
    """Build the per-core input maps (host-side shard + repack)."""
    x = np.ascontiguousarray(x, dtype=np.float32)
    warrs = [np.asarray(a, dtype=np.float32)
             for a in (car_w, cdr_w, cons1_w, cons2_w)]
    root_filler = np.asarray(root_filler, dtype=np.float32)
    root_role = np.asarray(root_role, dtype=np.float32)

    wts = np.zeros((W4, RP, RP), dtype=np.float32)
    for w, Wm in enumerate((D_l, D_r, E_l, E_r)):
        wts[w, :R, :R] = np.asarray(Wm, dtype=np.float32).T

    in_maps = []
    for c in range(NCORES):
        b0 = c * BLOC
        xs = x[b0:b0 + BLOC].reshape(BLOC * L, F * R)

        wblk = np.zeros((BLOC * L, 32), dtype=np.float32)
        pw = np.zeros((32, L), dtype=np.float32)
        for w in range(W4):
            for b in range(BLOC):
                wblk[b * L:(b + 1) * L, w * 8 + b] = warrs[w][b0 + b]
                pw[w * 8 + b] = warrs[w][b0 + b]

        rfrr = np.zeros((1, 4 * 128 + RP), dtype=np.float32)
        # out partition p = b2*64 + f within a batch pair
        rfrr[0, :512] = root_filler[b0:b0 + BLOC].reshape(512)
        rfrr[0, 512:512 + R] = root_role

        in_maps.append({"xs": xs, "wblk": wblk, "wts": wts,
                        "rfrr": rfrr, "pw": pw})
    return in_maps


def _unshuffle(arr):
    # (4, 2*F, R) with row = b2*64 + f  ->  (8, F, R), b = bp*2+b2
    return arr.reshape(BLOC, F, R)


def _assemble_outputs(results):
    car = np.concatenate(
        [_unshuffle(results[c]["out_car"]) for c in range(NCORES)], axis=0)
    cdr = np.concatenate(
        [_unshuffle(results[c]["out_cdr"]) for c in range(NCORES)], axis=0)
    cons = np.concatenate(
        [_unshuffle(results[c]["out_cons"]) for c in range(NCORES)], axis=0)
    ents = []
    maxes = []
    for w in range(W4):
        e = np.concatenate(
            [results[c]["out_stats"][w * 8:(w + 1) * 8, 0] for c in range(NCORES)])
        m = np.concatenate(
            [results[c]["out_stats"][w * 8:(w + 1) * 8, 1] for c in range(NCORES)])
        ents.append(e.astype(np.float32))
        maxes.append(m.astype(np.float32))
    return (car, cdr, cons) + tuple(ents) + tuple(maxes)


def _get_runner():
    global _runner
    with _runner_lock:
        if _runner is None:
            nc = _build_program()
            _runner = _make_executor(nc)
    return _runner


def _make_executor(nc):
    """Persistent jitted SPMD executor (adapted from bass2jax.run_bass_via_pjrt,
    hoisting the jit so repeated calls don't recompile)."""
    import jax
    import jax.numpy as jnp
    from jax.sharding import Mesh, PartitionSpec
    from jax.experimental.shard_map import shard_map
    import concourse.mybir as mybir
    from concourse import bass2jax

    bass2jax.install_neuronx_cc_hook()

    partition_name = (nc.partition_id_tensor.name
                      if nc.partition_id_tensor else None)
    in_names, out_names, out_avals, zero_outs = [], [], [], []
    for alloc in nc.m.functions[0].allocations:
        if not isinstance(alloc, mybir.MemoryLocationSet):
            continue
        name = alloc.memorylocations[0].name
        if alloc.kind == "ExternalInput":
            if name != partition_name:
                in_names.append(name)
        elif alloc.kind == "ExternalOutput":
            shape = tuple(alloc.tensor_shape)
            dtype = mybir.dt.np(alloc.dtype)
            out_names.append(name)
            out_avals.append(jax.core.ShapedArray(shape, dtype))
            zero_outs.append(np.zeros(shape, dtype))
    n_params = len(in_names)
    n_outs = len(out_avals)
    all_in_names = list(in_names) + list(out_names)
    if partition_name is not None:
        all_in_names.append(partition_name)

    donate = tuple(range(n_params, n_params + n_outs))

    def _body(*args):
        operands = list(args)
        if partition_name is not None:
            operands.append(bass2jax.partition_id_tensor())
        return tuple(bass2jax._bass_exec_p.bind(
            *operands,
            out_avals=tuple(out_avals),
            in_names=tuple(all_in_names),
            out_names=tuple(out_names),
            lowering_input_output_aliases=(),
            sim_require_finite=True,
            sim_require_nnan=True,
            nc=nc,
        ))

    devices = jax.devices()[:NCORES]
    mesh = Mesh(np.asarray(devices), ("core",))
    sharded = jax.jit(
        shard_map(_body, mesh=mesh,
                  in_specs=(PartitionSpec("core"),) * (n_params + n_outs),
                  out_specs=(PartitionSpec("core"),) * n_outs,
                  check_rep=False),
        donate_argnums=donate, keep_unused=True)

    class Executor:
        def __init__(self):
            self.in_names = in_names
            self.out_names = out_names
            self.zero_outs = zero_outs
            self.mesh = mesh
            self.sharded = sharded
            self.n_params = n_params

        def concat_inputs(self, in_maps):
            return [np.concatenate([np.asarray(in_maps[c][nm])
                                    for c in range(NCORES)], axis=0)
                    for nm in self.in_names]

        def fresh_zero_outs(self):
            return [np.zeros((NCORES * z.shape[0], *z.shape[1:]), z.dtype)
                    for z in self.zero_outs]

        def run(self, concat_in, concat_zeros):
            out_arrs = self.sharded(*concat_in, *concat_zeros)
            return [
                {nm: np.asarray(out_arrs[i]).reshape(NCORES, *zero_outs[i].shape)[c]
                 for i, nm in enumerate(self.out_names)}
                for c in range(NCORES)
            ]

    return Executor()


def kernel(**inputs):
    ex = _get_runner()
    in_maps = _prep_inputs(**inputs)
    results = ex.run(ex.concat_inputs(in_maps), ex.fresh_zero_outs())
    return _assemble_outputs(results)


# revision 24
# speedup vs baseline: 2574.7157x; 2574.7157x over previous
"""Trainium2 Bass kernel for nn_DiffTreeMachine (B=64, L=16, F=64, R=1023).

Data-parallel over batch: 8 NeuronCores x 8 batches each.

Per core, the computation is
  A_w[b]   = sum_l w_w[b,l] * x[b,l]          (4 weighted reductions, w in {car,cdr,cons1,cons2})
  car      = A_0 @ D_l^T
  cdr      = A_1 @ D_r^T
  cons     = A_2 @ E_l^T + A_3 @ E_r^T + outer(root_filler, root_role)
  ent_w[b] = -(sum_l p log(p+1e-12)) / log(L),  max_w[b] = max_l p

Device mapping:
  Stage 1: for each f (64) and r-chunk (8x128): matmul with x tile as the
    *stationary* operand and a sparse (128, 32) weight-block as the moving
    operand: out[(r),(w,b)] = x_slice.T @ wblk.  This lands A^T directly with
    r on partitions (the layout stage 2 needs for its contraction) with no
    separate transpose pass.  fp32 exact.
  Stage 2: out[(f,b2),(t)] = sum_rc Astage[rc].T @ W^T[rc]  accumulated in
    PSUM over 8 r-chunks, in float32r (PE full rate, ~11-bit-mantissa
    operand rounding, rel err ~1.5e-4).  The cons output accumulates two
    chains (E_l, E_r) plus a K=1 rank-1 matmul for the root term in one
    PSUM group.
"""

import math
import os
import sys
import threading

import numpy as np

for _p in ("/opt/trn_rl_repo", "/root/.axon_site/_ro/trn_rl_repo"):
    if os.path.isdir(_p) and _p not in sys.path:
        sys.path.insert(0, _p)

B, L, F, R = 64, 16, 64, 1023
NCORES = 8
BLOC = B // NCORES          # 8 batches per core
RP = 1024                   # R padded
NRC = 8                     # r-chunks of 128 (last has 127 real rows)
W4 = 4                      # four weight sets
ACOLS = W4 * BLOC * F       # astage cols: w*512 + b*64 + f

_runner_lock = threading.Lock()
_runner = None

# fp16 compute path: x / weight-block / W^T / root vectors stored and fed to
# the PE in fp16 (halves the dominant DMA traffic); PSUM accumulation stays
# fp32.  Measured end-to-end rel err ~5e-4 vs 1.4e-4 for the f32r path.
USE_FP16 = os.environ.get("KERNEL_FP16", "1") == "1"


def _build_program():
    import concourse.tile as tile
    import concourse.mybir as mybir
    from concourse import bacc

    f32 = mybir.dt.float32
    f32r = mybir.dt.float32r
    f16 = mybir.dt.float16
    xdt = f16 if USE_FP16 else f32      # x, wblk (stage-1 operands)
    wdt = f16 if USE_FP16 else f32r     # W^T, astage, rfrr (stage-2 operands)

    nc = bacc.Bacc("TRN2", target_bir_lowering=False, debug=False,
                   num_devices=NCORES)

    xs = nc.declare_dram_parameter("xs", [BLOC * L, F * R], xdt, isOutput=False)
    wblk = nc.declare_dram_parameter("wblk", [BLOC * L, 32], xdt, isOutput=False)
    # W^T padded on both dims: rows r->1024 (zero), cols t->1024 (zero)
    wts = nc.declare_dram_parameter("wts", [W4, RP, RP], wdt, isOutput=False)
    rfrr = nc.declare_dram_parameter("rfrr", [1, 4 * 128 + RP], wdt, isOutput=False)
    pw = nc.declare_dram_parameter("pw", [32, L], f32, isOutput=False)

# row layout: partition p = f*2 + b2 (b2 = batch index within the pair)
    out_car = nc.declare_dram_parameter("out_car", [4, F * 2, R], f32, isOutput=True)
    out_cdr = nc.declare_dram_parameter("out_cdr", [4, F * 2, R], f32, isOutput=True)
    out_cons = nc.declare_dram_parameter("out_cons", [4, F * 2, R], f32, isOutput=True)
    out_stats = nc.declare_dram_parameter("out_stats", [32, 2], f32, isOutput=True)

    outs = {"car": out_car, "cdr": out_cdr, "cons": out_cons}

    with tile.TileContext(nc) as tc:
        with (
            tc.tile_pool(name="const", bufs=1) as const,
            tc.tile_pool(name="astage", bufs=1) as apool,
            tc.tile_pool(name="xp", bufs=10) as xpool,
            tc.tile_pool(name="wtp", bufs=2) as wtpool,
            tc.tile_pool(name="obp", bufs=3) as obpool,
            tc.tile_pool(name="ps1", bufs=1, space="PSUM") as ps1,
            tc.tile_pool(name="ps2", bufs=3, space="PSUM") as ps2,
        ):
            wblk_sb = const.tile([BLOC * L, 32], xdt)
            nc.sync.dma_start(wblk_sb[:], wblk[:])
            rfrr_sb = const.tile([1, 4 * 128 + RP], wdt)
            nc.sync.dma_start(rfrr_sb[:], rfrr[:])
            pw_sb = const.tile([32, L], f32)
            nc.sync.dma_start(pw_sb[:], pw[:])

            # ---- stats: entropy + max of the four weight matrices ----
            eps = const.tile([32, 1], f32)
            nc.vector.memset(eps[:], 1e-12)
            lnp = const.tile([32, L], f32)
            nc.scalar.activation(lnp[:], pw_sb[:],
                                 mybir.ActivationFunctionType.Ln, bias=eps[:])
            plnp = const.tile([32, L], f32)
            nc.vector.tensor_mul(plnp[:], pw_sb[:], lnp[:])
            st = const.tile([32, 2], f32)
            nc.vector.reduce_sum(st[:, 0:1], plnp[:], axis=mybir.AxisListType.X)
            nc.vector.tensor_scalar_mul(st[:, 0:1], st[:, 0:1], -1.0 / math.log(L))
            nc.vector.reduce_max(st[:, 1:2], pw_sb[:], axis=mybir.AxisListType.X)
            nc.sync.dma_start(out_stats[:], st[:])

            # ---- A^T staging: 8 r-chunk tiles, cols = w*512 + b*64 + f ----
            astage = []
            for rc in range(NRC):
                a_t = apool.tile([128, ACOLS], wdt, tag=f"a{rc}", name=f"astage{rc}")
                astage.append(a_t)
            # r = 1023 row (last row of last chunk) is never written by
            # stage 1; zero it so stage 2 reads 0 * 0 there.  DVE partition
            # base must be 32-aligned, so clear the whole last group (stage-1
            # copies then overwrite rows 96..126).
            _last = astage[NRC - 1][96:128, :]
            # (memset has no f32r lowering; bitcast through f32 there)
            nc.vector.memset(_last if USE_FP16 else _last.bitcast(f32), 0.0)

            # ---- stage 1 ----
            # loop f-blocks of 8; within a block do r-chunk halves of 4 so
            # only 4 PSUM banks are needed.
            for fb in range(8):
                x_ts = []
                for f8 in range(8):
                    f = fb * 8 + f8
                    x_t = xpool.tile([128, R], xdt, tag="x", name=f"x{f}")
                    nc.sync.dma_start(x_t[:], xs[:, f * R:(f + 1) * R])
                    x_ts.append(x_t)
                for rch in range(2):
                    pst = []
                    for rc2 in range(4):
                        p_t = ps1.tile([128, 256], f32, tag=f"p{rc2}",
                                       name=f"ps1_{rc2}")
                        pst.append(p_t)
                    for f8 in range(8):
                        for rc2 in range(4):
                            rc = rch * 4 + rc2
                            nr = 128 if rc < 7 else 127
                            nc.tensor.matmul(
                                pst[rc2][0:nr, f8 * 32:(f8 + 1) * 32],
                                x_ts[f8][:, rc * 128: rc * 128 + nr],
                                wblk_sb[:],
                                start=True, stop=True,
                            )
                    for rc2 in range(4):
                        rc = rch * 4 + rc2
                        nr = 128 if rc < 7 else 127
                        # psum cols are f8*32 + w*8 + b; astage cols are
                        # w*512 + b*64 + f.  One strided copy per w.
                        src = pst[rc2].rearrange("p (f8 w b) -> p w b f8",
                                                 w=W4, b=BLOC)
                        dst = astage[rc].rearrange("p (w b f) -> p w b f",
                                                   b=BLOC, f=F)
                        for w in range(W4):
                            nc.vector.tensor_copy(
                                dst[0:nr, w, :, fb * 8:(fb + 1) * 8],
                                src[0:nr, w, :, :],
                            )

            # ---- stage 2 ----
            # section -> list of (w_idx, wt_idx, tag_group)
            sections = [
                ("car", [(0, 0, "A")]),
                ("cdr", [(1, 1, "B")]),
                ("cons", [(2, 2, "A"), (3, 3, "B")]),
            ]
            for sec, chains in sections:
                od = outs[sec]
                for th in range(2):
                    t0 = th * 512
                    nt = 512 if th == 0 else R - 512  # real (unpadded) cols
                    wt_sb = {}
                    for (w, wi, tg) in chains:
                        for rc in range(NRC):
                            wt_t = wtpool.tile([128, 512], wdt,
                                               tag=f"{tg}{rc}",
                                               name=f"wt_{sec}_{th}_{w}_{rc}")
                            nc.sync.dma_start(
                                wt_t[:],
                                wts[wi, rc * 128:(rc + 1) * 128, t0:t0 + 512])
                            wt_sb[(w, rc)] = wt_t
                    n_mm = len(chains) * NRC + (1 if sec == "cons" else 0)
                    for bp in range(4):
                        acc = ps2.tile([128, 512], f32, tag="acc",
                                       name=f"acc_{sec}_{th}_{bp}")
                        mm = 0
                        for (w, wi, tg) in chains:
                            for rc in range(NRC):
                                lhsT = astage[rc][:, w * 512 + bp * 128:
                                                  w * 512 + (bp + 1) * 128]
                                nc.tensor.matmul(
                                    acc[:],
                                    lhsT,
                                    wt_sb[(w, rc)][:],
                                    start=(mm == 0), stop=(mm == n_mm - 1),
                                )
                                mm += 1
                        if sec == "cons":
                            nc.tensor.matmul(
                                acc[:],
                                rfrr_sb[0:1, bp * 128:(bp + 1) * 128],
                                rfrr_sb[0:1, 512 + t0: 512 + t0 + 512],
                                start=False, stop=True,
                            )
                        ob = obpool.tile([128, 512], f32, tag="ob",
                                         name=f"ob_{sec}_{th}_{bp}")
                        nc.vector.tensor_copy(ob[:], acc[:])
                        nc.sync.dma_start(od[bp, :, t0:t0 + nt], ob[:, 0:nt])

    nc.compile()
    return nc


def _prep_inputs(x, car_w, cdr_w, cons1_w, cons2_w, root_filler,
                 D_l, D_r, E_l, E_r, root_role):
    """Build the per-core input maps (host-side shard + repack)."""
    cdt = np.float16 if USE_FP16 else np.float32
    x = np.ascontiguousarray(x, dtype=cdt)
    warrs = [np.asarray(a, dtype=np.float32)
             for a in (car_w, cdr_w, cons1_w, cons2_w)]
    root_filler = np.asarray(root_filler, dtype=cdt)
    root_role = np.asarray(root_role, dtype=cdt)

    wts = np.zeros((W4, RP, RP), dtype=cdt)
    for w, Wm in enumerate((D_l, D_r, E_l, E_r)):
        wts[w, :R, :R] = np.asarray(Wm, dtype=np.float32).T

    in_maps = []
    for c in range(NCORES):
        b0 = c * BLOC
        xs = x[b0:b0 + BLOC].reshape(BLOC * L, F * R)

        wblk = np.zeros((BLOC * L, 32), dtype=cdt)
        pw = np.zeros((32, L), dtype=np.float32)
        for w in range(W4):
            for b in range(BLOC):
                wblk[b * L:(b + 1) * L, w * 8 + b] = warrs[w][b0 + b]
                pw[w * 8 + b] = warrs[w][b0 + b]

        rfrr = np.zeros((1, 4 * 128 + RP), dtype=cdt)
        # out partition p = b2*64 + f within a batch pair
        rfrr[0, :512] = root_filler[b0:b0 + BLOC].reshape(512)
        rfrr[0, 512:512 + R] = root_role

        in_maps.append({"xs": xs, "wblk": wblk, "wts": wts,
                        "rfrr": rfrr, "pw": pw})
    return in_maps


def _unshuffle(arr):
    # (4, 2*F, R) with row = b2*64 + f  ->  (8, F, R), b = bp*2+b2
    return arr.reshape(BLOC, F, R)


def _assemble_outputs(results):
    car = np.concatenate(
        [_unshuffle(results[c]["out_car"]) for c in range(NCORES)], axis=0)
    cdr = np.concatenate(
        [_unshuffle(results[c]["out_cdr"]) for c in range(NCORES)], axis=0)
    cons = np.concatenate(
        [_unshuffle(results[c]["out_cons"]) for c in range(NCORES)], axis=0)
    ents = []
    maxes = []
    for w in range(W4):
        e = np.concatenate(
            [results[c]["out_stats"][w * 8:(w + 1) * 8, 0] for c in range(NCORES)])
        m = np.concatenate(
            [results[c]["out_stats"][w * 8:(w + 1) * 8, 1] for c in range(NCORES)])
        ents.append(e.astype(np.float32))
        maxes.append(m.astype(np.float32))
    return (car, cdr, cons) + tuple(ents) + tuple(maxes)


def _get_runner():
    global _runner
    with _runner_lock:
        if _runner is None:
            nc = _build_program()
            _runner = _make_executor(nc)
    return _runner


def _make_executor(nc):
    """Persistent jitted SPMD executor (adapted from bass2jax.run_bass_via_pjrt,
    hoisting the jit so repeated calls don't recompile)."""
    import jax
    import jax.numpy as jnp
    from jax.sharding import Mesh, PartitionSpec
    from jax.experimental.shard_map import shard_map
    import concourse.mybir as mybir
    from concourse import bass2jax

    bass2jax.install_neuronx_cc_hook()

    partition_name = (nc.partition_id_tensor.name
                      if nc.partition_id_tensor else None)
    in_names, out_names, out_avals, zero_outs = [], [], [], []
    for alloc in nc.m.functions[0].allocations:
        if not isinstance(alloc, mybir.MemoryLocationSet):
            continue
        name = alloc.memorylocations[0].name
        if alloc.kind == "ExternalInput":
            if name != partition_name:
                in_names.append(name)
        elif alloc.kind == "ExternalOutput":
            shape = tuple(alloc.tensor_shape)
            dtype = mybir.dt.np(alloc.dtype)
            out_names.append(name)
            out_avals.append(jax.core.ShapedArray(shape, dtype))
            zero_outs.append(np.zeros(shape, dtype))
    n_params = len(in_names)
    n_outs = len(out_avals)
    all_in_names = list(in_names) + list(out_names)
    if partition_name is not None:
        all_in_names.append(partition_name)

    donate = tuple(range(n_params, n_params + n_outs))

    def _body(*args):
        operands = list(args)
        if partition_name is not None:
            operands.append(bass2jax.partition_id_tensor())
        return tuple(bass2jax._bass_exec_p.bind(
            *operands,
            out_avals=tuple(out_avals),
            in_names=tuple(all_in_names),
            out_names=tuple(out_names),
            lowering_input_output_aliases=(),
            sim_require_finite=True,
            sim_require_nnan=True,
            nc=nc,
        ))

    devices = jax.devices()[:NCORES]
    mesh = Mesh(np.asarray(devices), ("core",))
    sharded = jax.jit(
        shard_map(_body, mesh=mesh,
                  in_specs=(PartitionSpec("core"),) * (n_params + n_outs),
                  out_specs=(PartitionSpec("core"),) * n_outs,
                  check_rep=False),
        donate_argnums=donate, keep_unused=True)

    class Executor:
        def __init__(self):
            self.in_names = in_names
            self.out_names = out_names
            self.zero_outs = zero_outs
            self.mesh = mesh
            self.sharded = sharded
            self.n_params = n_params
            self.body = _body
            self.n_outs = n_outs

        def make_chained(self, n):
            """jit that runs the kernel n times back-to-back on device,
            feeding iteration i's outputs as iteration i+1's output buffers.
            Used for timing (amortizes per-call dispatch overhead)."""
            def body_n(*args):
                ins = args[:n_params]
                cur = args[n_params:]
                for _ in range(n):
                    cur = _body(*ins, *cur)
                return cur
            return jax.jit(
                shard_map(body_n, mesh=mesh,
                          in_specs=(PartitionSpec("core"),) * (n_params + n_outs),
                          out_specs=(PartitionSpec("core"),) * n_outs,
                          check_rep=False),
                donate_argnums=donate, keep_unused=True)

        def concat_inputs(self, in_maps):
            return [np.concatenate([np.asarray(in_maps[c][nm])
                                    for c in range(NCORES)], axis=0)
                    for nm in self.in_names]

        def fresh_zero_outs(self):
            return [np.zeros((NCORES * z.shape[0], *z.shape[1:]), z.dtype)
                    for z in self.zero_outs]

        def run(self, concat_in, concat_zeros):
            out_arrs = self.sharded(*concat_in, *concat_zeros)
            return [
                {nm: np.asarray(out_arrs[i]).reshape(NCORES, *zero_outs[i].shape)[c]
                 for i, nm in enumerate(self.out_names)}
                for c in range(NCORES)
            ]

    return Executor()


def kernel(**inputs):
    ex = _get_runner()
    in_maps = _prep_inputs(**inputs)
    results = ex.run(ex.concat_inputs(in_maps), ex.fresh_zero_outs())
    return _assemble_outputs(results)


# revision 26
# speedup vs baseline: 2695.7424x; 1.0470x over previous
"""Trainium2 Bass kernel for nn_DiffTreeMachine (B=64, L=16, F=64, R=1023).

Data-parallel over batch: 8 NeuronCores x 8 batches each.

Per core, the computation is
  A_w[b]   = sum_l w_w[b,l] * x[b,l]          (4 weighted reductions, w in {car,cdr,cons1,cons2})
  car      = A_0 @ D_l^T
  cdr      = A_1 @ D_r^T
  cons     = A_2 @ E_l^T + A_3 @ E_r^T + outer(root_filler, root_role)
  ent_w[b] = -(sum_l p log(p+1e-12)) / log(L),  max_w[b] = max_l p

Device mapping:
  Stage 1: for each f (64) and r-chunk (8x128): matmul with x tile as the
    *stationary* operand and a sparse (128, 32) weight-block as the moving
    operand: out[(r),(w,b)] = x_slice.T @ wblk.  This lands A^T directly with
    r on partitions (the layout stage 2 needs for its contraction) with no
    separate transpose pass.  fp32 exact.
  Stage 2: out[(f,b2),(t)] = sum_rc Astage[rc].T @ W^T[rc]  accumulated in
    PSUM over 8 r-chunks, in float32r (PE full rate, ~11-bit-mantissa
    operand rounding, rel err ~1.5e-4).  The cons output accumulates two
    chains (E_l, E_r) plus a K=1 rank-1 matmul for the root term in one
    PSUM group.
"""

import math
import os
import sys
import threading

import numpy as np

for _p in ("/opt/trn_rl_repo", "/root/.axon_site/_ro/trn_rl_repo"):
    if os.path.isdir(_p) and _p not in sys.path:
        sys.path.insert(0, _p)

B, L, F, R = 64, 16, 64, 1023
NCORES = 8
BLOC = B // NCORES          # 8 batches per core
RP = 1024                   # R padded
NRC = 8                     # r-chunks of 128 (last has 127 real rows)
W4 = 4                      # four weight sets
ACOLS = W4 * BLOC * F       # astage cols: w*512 + b*64 + f

_runner_lock = threading.Lock()
_runner = None

# fp16 compute path: x / weight-block / W^T / root vectors stored and fed to
# the PE in fp16 (halves the dominant DMA traffic); PSUM accumulation stays
# fp32.  Measured end-to-end rel err ~5e-4 vs 1.4e-4 for the f32r path.
USE_FP16 = os.environ.get("KERNEL_FP16", "1") == "1"


def _build_program():
    import concourse.tile as tile
    import concourse.mybir as mybir
    from concourse import bacc

    f32 = mybir.dt.float32
    f32r = mybir.dt.float32r
    f16 = mybir.dt.float16
    xdt = f16 if USE_FP16 else f32      # x, wblk (stage-1 operands)
    wdt = f16 if USE_FP16 else f32r     # W^T, astage, rfrr (stage-2 operands)

    nc = bacc.Bacc("TRN2", target_bir_lowering=False, debug=False,
                   num_devices=NCORES)

    xs = nc.declare_dram_parameter("xs", [BLOC * L, F * R], xdt, isOutput=False)
    wblk = nc.declare_dram_parameter("wblk", [BLOC * L, 32], xdt, isOutput=False)
    # W^T padded on both dims: rows r->1024 (zero), cols t->1024 (zero)
    wts = nc.declare_dram_parameter("wts", [W4, RP, RP], wdt, isOutput=False)
    rfrr = nc.declare_dram_parameter("rfrr", [1, 4 * 128 + RP], wdt, isOutput=False)
    pw = nc.declare_dram_parameter("pw", [32, L], f32, isOutput=False)

# row layout: partition p = f*2 + b2 (b2 = batch index within the pair)
    out_car = nc.declare_dram_parameter("out_car", [4, F * 2, R], f32, isOutput=True)
    out_cdr = nc.declare_dram_parameter("out_cdr", [4, F * 2, R], f32, isOutput=True)
    out_cons = nc.declare_dram_parameter("out_cons", [4, F * 2, R], f32, isOutput=True)
    out_stats = nc.declare_dram_parameter("out_stats", [32, 2], f32, isOutput=True)

    outs = {"car": out_car, "cdr": out_cdr, "cons": out_cons}

    with tile.TileContext(nc) as tc:
        with (
            tc.tile_pool(name="const", bufs=1) as const,
            tc.tile_pool(name="astage", bufs=1) as apool,
            tc.tile_pool(name="xp", bufs=6) as xpool,
            tc.tile_pool(name="wtp", bufs=1) as wtpool,
            tc.tile_pool(name="obp", bufs=2) as obpool,
            tc.tile_pool(name="ps1", bufs=1, space="PSUM") as ps1,
            tc.tile_pool(name="ps2", bufs=3, space="PSUM") as ps2,
        ):
            wblk_sb = const.tile([BLOC * L, 32], xdt)
            nc.sync.dma_start(wblk_sb[:], wblk[:])
            rfrr_sb = const.tile([1, 4 * 128 + RP], wdt)
            nc.sync.dma_start(rfrr_sb[:], rfrr[:])
            pw_sb = const.tile([32, L], f32)
            nc.sync.dma_start(pw_sb[:], pw[:])

            # ---- stats: entropy + max of the four weight matrices ----
            eps = const.tile([32, 1], f32)
            nc.vector.memset(eps[:], 1e-12)
            lnp = const.tile([32, L], f32)
            nc.scalar.activation(lnp[:], pw_sb[:],
                                 mybir.ActivationFunctionType.Ln, bias=eps[:])
            plnp = const.tile([32, L], f32)
            nc.vector.tensor_mul(plnp[:], pw_sb[:], lnp[:])
            st = const.tile([32, 2], f32)
            nc.vector.reduce_sum(st[:, 0:1], plnp[:], axis=mybir.AxisListType.X)
            nc.vector.tensor_scalar_mul(st[:, 0:1], st[:, 0:1], -1.0 / math.log(L))
            nc.vector.reduce_max(st[:, 1:2], pw_sb[:], axis=mybir.AxisListType.X)
            nc.sync.dma_start(out_stats[:], st[:])

            # ---- A^T staging: 8 r-chunk tiles, cols = w*512 + b*64 + f ----
            astage = []
            for rc in range(NRC):
                a_t = apool.tile([128, ACOLS], wdt, tag=f"a{rc}", name=f"astage{rc}")
                astage.append(a_t)
            # r = 1023 row (last row of last chunk) is never written by
            # stage 1; zero it so stage 2 reads 0 * 0 there.  DVE partition
            # base must be 32-aligned, so clear the whole last group (stage-1
            # copies then overwrite rows 96..126).
            _last = astage[NRC - 1][96:128, :]
            # (memset has no f32r lowering; bitcast through f32 there)
            nc.vector.memset(_last if USE_FP16 else _last.bitcast(f32), 0.0)

            # ---- W^T loads: one DMA per matrix, all resident ----
            # wts[w] rows are rc*128 + r; land as (r partitions, rc*1024 + t)
            wt_sb = []
            for w in range(W4):
                wt_t = wtpool.tile([128, NRC * RP], wdt, tag=f"wt{w}",
                                   name=f"wt{w}")
                nc.sync.dma_start(
                    wt_t[:].rearrange("p (rc t) -> p rc t", rc=NRC),
                    wts[w].rearrange("(rc r) t -> r rc t", r=128),
                )
                wt_sb.append(wt_t)

            # ---- stage 1 ----
            # f-blocks of 16; r-chunk halves of 4 so stage 1 uses only 4
            # PSUM banks.  x tiles carry 4 f-slices per DMA.
            for fb in range(4):
                x_ts = []
                for x4 in range(4):
                    f0 = fb * 16 + x4 * 4
                    x_t = xpool.tile([128, 4 * R], xdt, tag="x",
                                     name=f"x{f0}")
                    nc.sync.dma_start(x_t[:], xs[:, f0 * R:(f0 + 4) * R])
                    x_ts.append(x_t)
                for rch in range(2):
                    pst = []
                    for rc2 in range(4):
                        p_t = ps1.tile([128, 512], f32, tag=f"p{rc2}",
                                       name=f"ps1_{rc2}")
                        pst.append(p_t)
                    for f16 in range(16):
                        xsl = x_ts[f16 // 4]
                        xoff = (f16 % 4) * R
                        for rc2 in range(4):
                            rc = rch * 4 + rc2
                            nr = 128 if rc < 7 else 127
                            nc.tensor.matmul(
                                pst[rc2][0:nr, f16 * 32:(f16 + 1) * 32],
                                xsl[:, xoff + rc * 128: xoff + rc * 128 + nr],
                                wblk_sb[:],
                                start=True, stop=True,
                            )
                    for rc2 in range(4):
                        rc = rch * 4 + rc2
                        nr = 128 if rc < 7 else 127
                        # psum cols are f16*32 + w*8 + b; astage cols are
                        # w*512 + b*64 + f.  One strided copy per chunk.
                        src = pst[rc2].rearrange("p (f16 w b) -> p w b f16",
                                                 w=W4, b=BLOC)
                        dst = astage[rc].rearrange("p (w b f) -> p w b f",
                                                   b=BLOC, f=F)
                        nc.vector.tensor_copy(
                            dst[0:nr, :, :, fb * 16:(fb + 1) * 16],
                            src[0:nr, :, :, :],
                        )

            # ---- stage 2 ----
            # section -> list of w indices to accumulate
            sections = [("car", [0]), ("cdr", [1]), ("cons", [2, 3])]
            for sec, chain in sections:
                od = outs[sec].rearrange("bp p t -> p bp t")
                ob = obpool.tile([128, 4 * RP], f32, tag="ob",
                                 name=f"ob_{sec}")
                n_mm = len(chain) * NRC + (1 if sec == "cons" else 0)
                for bp in range(4):
                    for th in range(2):
                        t0 = th * 512
                        acc = ps2.tile([128, 512], f32, tag="acc",
                                       name=f"acc_{sec}_{bp}_{th}")
                        mm = 0
                        for w in chain:
                            for rc in range(NRC):
                                lhsT = astage[rc][:, w * 512 + bp * 128:
                                                  w * 512 + (bp + 1) * 128]
                                nc.tensor.matmul(
                                    acc[:],
                                    lhsT,
                                    wt_sb[w][:, rc * RP + t0: rc * RP + t0 + 512],
                                    start=(mm == 0), stop=(mm == n_mm - 1),
                                )
                                mm += 1
                        if sec == "cons":
                            nc.tensor.matmul(
                                acc[:],
                                rfrr_sb[0:1, bp * 128:(bp + 1) * 128],
                                rfrr_sb[0:1, 512 + t0: 512 + t0 + 512],
                                start=False, stop=True,
                            )
                        nc.vector.tensor_copy(
                            ob[:, bp * RP + t0: bp * RP + t0 + 512], acc[:])
                # one DMA per output tensor: (p, bp, t) <- ob cols bp*1024+t
                nc.sync.dma_start(
                    od[:],
                    ob[:].rearrange("p (bp t) -> p bp t", bp=4)[:, :, 0:R],
                )

    nc.compile()
    return nc


def _prep_inputs(x, car_w, cdr_w, cons1_w, cons2_w, root_filler,
                 D_l, D_r, E_l, E_r, root_role):
    """Build the per-core input maps (host-side shard + repack)."""
    cdt = np.float16 if USE_FP16 else np.float32
    x = np.ascontiguousarray(x, dtype=cdt)
    warrs = [np.asarray(a, dtype=np.float32)
             for a in (car_w, cdr_w, cons1_w, cons2_w)]
    root_filler = np.asarray(root_filler, dtype=cdt)
    root_role = np.asarray(root_role, dtype=cdt)

    wts = np.zeros((W4, RP, RP), dtype=cdt)
    for w, Wm in enumerate((D_l, D_r, E_l, E_r)):
        wts[w, :R, :R] = np.asarray(Wm, dtype=np.float32).T

    in_maps = []
    for c in range(NCORES):
        b0 = c * BLOC
        xs = x[b0:b0 + BLOC].reshape(BLOC * L, F * R)

        wblk = np.zeros((BLOC * L, 32), dtype=cdt)
        pw = np.zeros((32, L), dtype=np.float32)
        for w in range(W4):
            for b in range(BLOC):
                wblk[b * L:(b + 1) * L, w * 8 + b] = warrs[w][b0 + b]
                pw[w * 8 + b] = warrs[w][b0 + b]

        rfrr = np.zeros((1, 4 * 128 + RP), dtype=cdt)
        # out partition p = b2*64 + f within a batch pair
        rfrr[0, :512] = root_filler[b0:b0 + BLOC].reshape(512)
        rfrr[0, 512:512 + R] = root_role

        in_maps.append({"xs": xs, "wblk": wblk, "wts": wts,
                        "rfrr": rfrr, "pw": pw})
    return in_maps


def _unshuffle(arr):
    # (4, 2*F, R) with row = b2*64 + f  ->  (8, F, R), b = bp*2+b2
    return arr.reshape(BLOC, F, R)


def _assemble_outputs(results):
    car = np.concatenate(
        [_unshuffle(results[c]["out_car"]) for c in range(NCORES)], axis=0)
    cdr = np.concatenate(
        [_unshuffle(results[c]["out_cdr"]) for c in range(NCORES)], axis=0)
    cons = np.concatenate(
        [_unshuffle(results[c]["out_cons"]) for c in range(NCORES)], axis=0)
    ents = []
    maxes = []
    for w in range(W4):
        e = np.concatenate(
            [results[c]["out_stats"][w * 8:(w + 1) * 8, 0] for c in range(NCORES)])
        m = np.concatenate(
            [results[c]["out_stats"][w * 8:(w + 1) * 8, 1] for c in range(NCORES)])
        ents.append(e.astype(np.float32))
        maxes.append(m.astype(np.float32))
    return (car, cdr, cons) + tuple(ents) + tuple(maxes)


def _get_runner():
    global _runner
    with _runner_lock:
        if _runner is None:
            nc = _build_program()
            _runner = _make_executor(nc)
    return _runner


def _make_executor(nc):
    """Persistent jitted SPMD executor (adapted from bass2jax.run_bass_via_pjrt,
    hoisting the jit so repeated calls don't recompile)."""
    import jax
    import jax.numpy as jnp
    from jax.sharding import Mesh, PartitionSpec
    from jax.experimental.shard_map import shard_map
    import concourse.mybir as mybir
    from concourse import bass2jax

    bass2jax.install_neuronx_cc_hook()

    partition_name = (nc.partition_id_tensor.name
                      if nc.partition_id_tensor else None)
    in_names, out_names, out_avals, zero_outs = [], [], [], []
    for alloc in nc.m.functions[0].allocations:
        if not isinstance(alloc, mybir.MemoryLocationSet):
            continue
        name = alloc.memorylocations[0].name
        if alloc.kind == "ExternalInput":
            if name != partition_name:
                in_names.append(name)
        elif alloc.kind == "ExternalOutput":
            shape = tuple(alloc.tensor_shape)
            dtype = mybir.dt.np(alloc.dtype)
            out_names.append(name)
            out_avals.append(jax.core.ShapedArray(shape, dtype))
            zero_outs.append(np.zeros(shape, dtype))
    n_params = len(in_names)
    n_outs = len(out_avals)
    all_in_names = list(in_names) + list(out_names)
    if partition_name is not None:
        all_in_names.append(partition_name)

    donate = tuple(range(n_params, n_params + n_outs))

    def _body(*args):
        operands = list(args)
        if partition_name is not None:
            operands.append(bass2jax.partition_id_tensor())
        return tuple(bass2jax._bass_exec_p.bind(
            *operands,
            out_avals=tuple(out_avals),
            in_names=tuple(all_in_names),
            out_names=tuple(out_names),
            lowering_input_output_aliases=(),
            sim_require_finite=True,
            sim_require_nnan=True,
            nc=nc,
        ))

    devices = jax.devices()[:NCORES]
    mesh = Mesh(np.asarray(devices), ("core",))
    sharded = jax.jit(
        shard_map(_body, mesh=mesh,
                  in_specs=(PartitionSpec("core"),) * (n_params + n_outs),
                  out_specs=(PartitionSpec("core"),) * n_outs,
                  check_rep=False),
        donate_argnums=donate, keep_unused=True)

    class Executor:
        def __init__(self):
            self.in_names = in_names
            self.out_names = out_names
            self.zero_outs = zero_outs
            self.mesh = mesh
            self.sharded = sharded
            self.n_params = n_params
            self.body = _body
            self.n_outs = n_outs

        def make_chained(self, n):
            """jit that runs the kernel n times back-to-back on device,
            feeding iteration i's outputs as iteration i+1's output buffers.
            Used for timing (amortizes per-call dispatch overhead)."""
            def body_n(*args):
                ins = args[:n_params]
                cur = args[n_params:]
                for _ in range(n):
                    cur = _body(*ins, *cur)
                return cur
            return jax.jit(
                shard_map(body_n, mesh=mesh,
                          in_specs=(PartitionSpec("core"),) * (n_params + n_outs),
                          out_specs=(PartitionSpec("core"),) * n_outs,
                          check_rep=False),
                donate_argnums=donate, keep_unused=True)

        def concat_inputs(self, in_maps):
            return [np.concatenate([np.asarray(in_maps[c][nm])
                                    for c in range(NCORES)], axis=0)
                    for nm in self.in_names]

        def fresh_zero_outs(self):
            return [np.zeros((NCORES * z.shape[0], *z.shape[1:]), z.dtype)
                    for z in self.zero_outs]

        def run(self, concat_in, concat_zeros):
            out_arrs = self.sharded(*concat_in, *concat_zeros)
            return [
                {nm: np.asarray(out_arrs[i]).reshape(NCORES, *zero_outs[i].shape)[c]
                 for i, nm in enumerate(self.out_names)}
                for c in range(NCORES)
            ]

    return Executor()


def kernel(**inputs):
    ex = _get_runner()
    in_maps = _prep_inputs(**inputs)
    results = ex.run(ex.concat_inputs(in_maps), ex.fresh_zero_outs())
    return _assemble_outputs(results)


# revision 27
# speedup vs baseline: 3123.2006x; 1.1586x over previous
"""Trainium2 Bass kernel for nn_DiffTreeMachine (B=64, L=16, F=64, R=1023).

Data-parallel over batch: 8 NeuronCores x 8 batches each.

Per core, the computation is
  A_w[b]   = sum_l w_w[b,l] * x[b,l]          (4 weighted reductions, w in {car,cdr,cons1,cons2})
  car      = A_0 @ D_l^T
  cdr      = A_1 @ D_r^T
  cons     = A_2 @ E_l^T + A_3 @ E_r^T + outer(root_filler, root_role)
  ent_w[b] = -(sum_l p log(p+1e-12)) / log(L),  max_w[b] = max_l p

Device mapping:
  Stage 1: for each f (64) and r-chunk (8x128): matmul with x tile as the
    *stationary* operand and a sparse (128, 32) weight-block as the moving
    operand: out[(r),(w,b)] = x_slice.T @ wblk.  This lands A^T directly with
    r on partitions (the layout stage 2 needs for its contraction) with no
    separate transpose pass.  fp32 exact.
  Stage 2: out[(f,b2),(t)] = sum_rc Astage[rc].T @ W^T[rc]  accumulated in
    PSUM over 8 r-chunks, in float32r (PE full rate, ~11-bit-mantissa
    operand rounding, rel err ~1.5e-4).  The cons output accumulates two
    chains (E_l, E_r) plus a K=1 rank-1 matmul for the root term in one
    PSUM group.
"""

import math
import os
import sys
import threading

import numpy as np

for _p in ("/opt/trn_rl_repo", "/root/.axon_site/_ro/trn_rl_repo"):
    if os.path.isdir(_p) and _p not in sys.path:
        sys.path.insert(0, _p)

B, L, F, R = 64, 16, 64, 1023
NCORES = 8
BLOC = B // NCORES          # 8 batches per core
RP = 1024                   # R padded
NRC = 8                     # r-chunks of 128 (last has 127 real rows)
W4 = 4                      # four weight sets
ACOLS = W4 * BLOC * F       # astage cols: w*512 + b*64 + f

_runner_lock = threading.Lock()
_runner = None

# fp16 compute path: x / weight-block / W^T / root vectors stored and fed to
# the PE in fp16 (halves the dominant DMA traffic); PSUM accumulation stays
# fp32.  Measured end-to-end rel err ~5e-4 vs 1.4e-4 for the f32r path.
USE_FP16 = os.environ.get("KERNEL_FP16", "1") == "1"


def _build_program():
    import concourse.tile as tile
    import concourse.mybir as mybir
    from concourse import bacc

    f32 = mybir.dt.float32
    f32r = mybir.dt.float32r
    f16 = mybir.dt.float16
    xdt = f16 if USE_FP16 else f32      # x, wblk (stage-1 operands)
    wdt = f16 if USE_FP16 else f32r     # W^T, astage, rfrr (stage-2 operands)

    nc = bacc.Bacc("TRN2", target_bir_lowering=False, debug=False,
                   num_devices=NCORES)

    xs = nc.declare_dram_parameter("xs", [BLOC * L, F * R], xdt, isOutput=False)
    wblk = nc.declare_dram_parameter("wblk", [BLOC * L, 32], xdt, isOutput=False)
    # W^T padded on both dims: rows r->1024 (zero), cols t->1024 (zero)
    wts = nc.declare_dram_parameter("wts", [W4, RP, RP], wdt, isOutput=False)
    rfrr = nc.declare_dram_parameter("rfrr", [1, 4 * 128 + RP], wdt, isOutput=False)
    pw = nc.declare_dram_parameter("pw", [32, L], f32, isOutput=False)

# row layout: partition p = f*2 + b2 (b2 = batch index within the pair)
    out_car = nc.declare_dram_parameter("out_car", [4, F * 2, R], f32, isOutput=True)
    out_cdr = nc.declare_dram_parameter("out_cdr", [4, F * 2, R], f32, isOutput=True)
    out_cons = nc.declare_dram_parameter("out_cons", [4, F * 2, R], f32, isOutput=True)
    out_stats = nc.declare_dram_parameter("out_stats", [32, 2], f32, isOutput=True)

    outs = {"car": out_car, "cdr": out_cdr, "cons": out_cons}

    with tile.TileContext(nc) as tc:
        with (
            tc.tile_pool(name="const", bufs=1) as const,
            tc.tile_pool(name="astage", bufs=1) as apool,
            tc.tile_pool(name="xp", bufs=6) as xpool,
            tc.tile_pool(name="wtp", bufs=1) as wtpool,
            tc.tile_pool(name="obp", bufs=2) as obpool,
            tc.tile_pool(name="ps1", bufs=1, space="PSUM") as ps1,
            tc.tile_pool(name="ps2", bufs=3, space="PSUM") as ps2,
        ):
            wblk_sb = const.tile([BLOC * L, 32], xdt)
            nc.sync.dma_start(wblk_sb[:], wblk[:])
            rfrr_sb = const.tile([1, 4 * 128 + RP], wdt)
            nc.sync.dma_start(rfrr_sb[:], rfrr[:])
            pw_sb = const.tile([32, L], f32)
            nc.sync.dma_start(pw_sb[:], pw[:])

            # ---- stats: entropy + max of the four weight matrices ----
            eps = const.tile([32, 1], f32)
            nc.vector.memset(eps[:], 1e-12)
            lnp = const.tile([32, L], f32)
            nc.scalar.activation(lnp[:], pw_sb[:],
                                 mybir.ActivationFunctionType.Ln, bias=eps[:])
            plnp = const.tile([32, L], f32)
            nc.vector.tensor_mul(plnp[:], pw_sb[:], lnp[:])
            st = const.tile([32, 2], f32)
            nc.vector.reduce_sum(st[:, 0:1], plnp[:], axis=mybir.AxisListType.X)
            nc.vector.tensor_scalar_mul(st[:, 0:1], st[:, 0:1], -1.0 / math.log(L))
            nc.vector.reduce_max(st[:, 1:2], pw_sb[:], axis=mybir.AxisListType.X)
            nc.sync.dma_start(out_stats[:], st[:])

            # ---- A^T staging: 8 r-chunk tiles, cols = w*512 + b*64 + f ----
            astage = []
            for rc in range(NRC):
                a_t = apool.tile([128, ACOLS], wdt, tag=f"a{rc}", name=f"astage{rc}")
                astage.append(a_t)
            # r = 1023 row (last row of last chunk) is never written by
            # stage 1; zero it so stage 2 reads 0 * 0 there.  DVE partition
            # base must be 32-aligned, so clear the whole last group (stage-1
            # copies then overwrite rows 96..126).
            _last = astage[NRC - 1][96:128, :]
            # (memset has no f32r lowering; bitcast through f32 there)
            nc.vector.memset(_last if USE_FP16 else _last.bitcast(f32), 0.0)

            # ---- stage 1 ----
            # f-blocks of 16; r-chunk halves of 4 so stage 1 uses only 4
            # PSUM banks.  x tiles carry 4 f-slices per DMA.
            for fb in range(4):
                x_ts = []
                for x4 in range(4):
                    f0 = fb * 16 + x4 * 4
                    x_t = xpool.tile([128, 4 * R], xdt, tag="x",
                                     name=f"x{f0}")
                    nc.sync.dma_start(x_t[:], xs[:, f0 * R:(f0 + 4) * R])
                    x_ts.append(x_t)
                for rch in range(2):
                    pst = []
                    for rc2 in range(4):
                        p_t = ps1.tile([128, 512], f32, tag=f"p{rc2}",
                                       name=f"ps1_{rc2}")
                        pst.append(p_t)
                    for f16 in range(16):
                        xsl = x_ts[f16 // 4]
                        xoff = (f16 % 4) * R
                        for rc2 in range(4):
                            rc = rch * 4 + rc2
                            nr = 128 if rc < 7 else 127
                            nc.tensor.matmul(
                                pst[rc2][0:nr, f16 * 32:(f16 + 1) * 32],
                                xsl[:, xoff + rc * 128: xoff + rc * 128 + nr],
                                wblk_sb[:],
                                start=True, stop=True,
                            )
                    for rc2 in range(4):
                        rc = rch * 4 + rc2
                        nr = 128 if rc < 7 else 127
                        # psum cols are f16*32 + w*8 + b; astage cols are
                        # w*512 + b*64 + f.  One strided copy per chunk.
                        src = pst[rc2].rearrange("p (f16 w b) -> p w b f16",
                                                 w=W4, b=BLOC)
                        dst = astage[rc].rearrange("p (w b f) -> p w b f",
                                                   b=BLOC, f=F)
                        nc.vector.tensor_copy(
                            dst[0:nr, :, :, fb * 16:(fb + 1) * 16],
                            src[0:nr, :, :, :],
                        )

            # ---- W^T loads: one DMA per matrix, all resident ----
            # wts[w] rows are rc*128 + r; land as (r partitions, rc*1024 + t)
            wt_sb = []
            for w in range(W4):
                wt_t = wtpool.tile([128, NRC * RP], wdt, tag=f"wt{w}",
                                   name=f"wt{w}")
                nc.sync.dma_start(
                    wt_t[:].rearrange("p (rc t) -> p rc t", rc=NRC),
                    wts[w].rearrange("(rc r) t -> r rc t", r=128),
                )
                wt_sb.append(wt_t)

            # ---- stage 2 ----
            # section -> list of w indices to accumulate
            sections = [("car", [0]), ("cdr", [1]), ("cons", [2, 3])]
            for sec, chain in sections:
                od = outs[sec].rearrange("bp p t -> p bp t")
                ob = obpool.tile([128, 4 * RP], f32, tag="ob",
                                 name=f"ob_{sec}")
                n_mm = len(chain) * NRC + (1 if sec == "cons" else 0)
                for bp in range(4):
                    for th in range(2):
                        t0 = th * 512
                        acc = ps2.tile([128, 512], f32, tag="acc",
                                       name=f"acc_{sec}_{bp}_{th}")
                        mm = 0
                        for w in chain:
                            for rc in range(NRC):
                                lhsT = astage[rc][:, w * 512 + bp * 128:
                                                  w * 512 + (bp + 1) * 128]
                                nc.tensor.matmul(
                                    acc[:],
                                    lhsT,
                                    wt_sb[w][:, rc * RP + t0: rc * RP + t0 + 512],
                                    start=(mm == 0), stop=(mm == n_mm - 1),
                                )
                                mm += 1
                        if sec == "cons":
                            nc.tensor.matmul(
                                acc[:],
                                rfrr_sb[0:1, bp * 128:(bp + 1) * 128],
                                rfrr_sb[0:1, 512 + t0: 512 + t0 + 512],
                                start=False, stop=True,
                            )
                        nc.vector.tensor_copy(
                            ob[:, bp * RP + t0: bp * RP + t0 + 512], acc[:])
                # one DMA per output tensor: (p, bp, t) <- ob cols bp*1024+t
                nc.sync.dma_start(
                    od[:],
                    ob[:].rearrange("p (bp t) -> p bp t", bp=4)[:, :, 0:R],
                )

    nc.compile()
    return nc


def _prep_inputs(x, car_w, cdr_w, cons1_w, cons2_w, root_filler,
                 D_l, D_r, E_l, E_r, root_role):
    """Build the per-core input maps (host-side shard + repack)."""
    cdt = np.float16 if USE_FP16 else np.float32
    x = np.ascontiguousarray(x, dtype=cdt)
    warrs = [np.asarray(a, dtype=np.float32)
             for a in (car_w, cdr_w, cons1_w, cons2_w)]
    root_filler = np.asarray(root_filler, dtype=cdt)
    root_role = np.asarray(root_role, dtype=cdt)

    wts = np.zeros((W4, RP, RP), dtype=cdt)
    for w, Wm in enumerate((D_l, D_r, E_l, E_r)):
        wts[w, :R, :R] = np.asarray(Wm, dtype=np.float32).T

    in_maps = []
    for c in range(NCORES):
        b0 = c * BLOC
        xs = x[b0:b0 + BLOC].reshape(BLOC * L, F * R)

        wblk = np.zeros((BLOC * L, 32), dtype=cdt)
        pw = np.zeros((32, L), dtype=np.float32)
        for w in range(W4):
            for b in range(BLOC):
                wblk[b * L:(b + 1) * L, w * 8 + b] = warrs[w][b0 + b]
                pw[w * 8 + b] = warrs[w][b0 + b]

        rfrr = np.zeros((1, 4 * 128 + RP), dtype=cdt)
        # out partition p = b2*64 + f within a batch pair
        rfrr[0, :512] = root_filler[b0:b0 + BLOC].reshape(512)
        rfrr[0, 512:512 + R] = root_role

        in_maps.append({"xs": xs, "wblk": wblk, "wts": wts,
                        "rfrr": rfrr, "pw": pw})
    return in_maps


def _unshuffle(arr):
    # (4, 2*F, R) with row = b2*64 + f  ->  (8, F, R), b = bp*2+b2
    return arr.reshape(BLOC, F, R)


def _assemble_outputs(results):
    car = np.concatenate(
        [_unshuffle(results[c]["out_car"]) for c in range(NCORES)], axis=0)
    cdr = np.concatenate(
        [_unshuffle(results[c]["out_cdr"]) for c in range(NCORES)], axis=0)
    cons = np.concatenate(
        [_unshuffle(results[c]["out_cons"]) for c in range(NCORES)], axis=0)
    ents = []
    maxes = []
    for w in range(W4):
        e = np.concatenate(
            [results[c]["out_stats"][w * 8:(w + 1) * 8, 0] for c in range(NCORES)])
        m = np.concatenate(
            [results[c]["out_stats"][w * 8:(w + 1) * 8, 1] for c in range(NCORES)])
        ents.append(e.astype(np.float32))
        maxes.append(m.astype(np.float32))
    return (car, cdr, cons) + tuple(ents) + tuple(maxes)


def _get_runner():
    global _runner
    with _runner_lock:
        if _runner is None:
            nc = _build_program()
            _runner = _make_executor(nc)
    return _runner


def _make_executor(nc):
    """Persistent jitted SPMD executor (adapted from bass2jax.run_bass_via_pjrt,
    hoisting the jit so repeated calls don't recompile)."""
    import jax
    import jax.numpy as jnp
    from jax.sharding import Mesh, PartitionSpec
    from jax.experimental.shard_map import shard_map
    import concourse.mybir as mybir
    from concourse import bass2jax

    bass2jax.install_neuronx_cc_hook()

    partition_name = (nc.partition_id_tensor.name
                      if nc.partition_id_tensor else None)
    in_names, out_names, out_avals, zero_outs = [], [], [], []
    for alloc in nc.m.functions[0].allocations:
        if not isinstance(alloc, mybir.MemoryLocationSet):
            continue
        name = alloc.memorylocations[0].name
        if alloc.kind == "ExternalInput":
            if name != partition_name:
                in_names.append(name)
        elif alloc.kind == "ExternalOutput":
            shape = tuple(alloc.tensor_shape)
            dtype = mybir.dt.np(alloc.dtype)
            out_names.append(name)
            out_avals.append(jax.core.ShapedArray(shape, dtype))
            zero_outs.append(np.zeros(shape, dtype))
    n_params = len(in_names)
    n_outs = len(out_avals)
    all_in_names = list(in_names) + list(out_names)
    if partition_name is not None:
        all_in_names.append(partition_name)

    donate = tuple(range(n_params, n_params + n_outs))

    def _body(*args):
        operands = list(args)
        if partition_name is not None:
            operands.append(bass2jax.partition_id_tensor())
        return tuple(bass2jax._bass_exec_p.bind(
            *operands,
            out_avals=tuple(out_avals),
            in_names=tuple(all_in_names),
            out_names=tuple(out_names),
            lowering_input_output_aliases=(),
            sim_require_finite=True,
            sim_require_nnan=True,
            nc=nc,
        ))

    devices = jax.devices()[:NCORES]
    mesh = Mesh(np.asarray(devices), ("core",))
    sharded = jax.jit(
        shard_map(_body, mesh=mesh,
                  in_specs=(PartitionSpec("core"),) * (n_params + n_outs),
                  out_specs=(PartitionSpec("core"),) * n_outs,
                  check_rep=False),
        donate_argnums=donate, keep_unused=True)

    class Executor:
        def __init__(self):
            self.in_names = in_names
            self.out_names = out_names
            self.zero_outs = zero_outs
            self.mesh = mesh
            self.sharded = sharded
            self.n_params = n_params
            self.body = _body
            self.n_outs = n_outs

        def make_chained(self, n):
            """jit that runs the kernel n times back-to-back on device,
            feeding iteration i's outputs as iteration i+1's output buffers.
            Used for timing (amortizes per-call dispatch overhead)."""
            def body_n(*args):
                ins = args[:n_params]
                cur = args[n_params:]
                for _ in range(n):
                    cur = _body(*ins, *cur)
                return cur
            return jax.jit(
                shard_map(body_n, mesh=mesh,
                          in_specs=(PartitionSpec("core"),) * (n_params + n_outs),
                          out_specs=(PartitionSpec("core"),) * n_outs,
                          check_rep=False),
                donate_argnums=donate, keep_unused=True)

        def concat_inputs(self, in_maps):
            return [np.concatenate([np.asarray(in_maps[c][nm])
                                    for c in range(NCORES)], axis=0)
                    for nm in self.in_names]

        def fresh_zero_outs(self):
            return [np.zeros((NCORES * z.shape[0], *z.shape[1:]), z.dtype)
                    for z in self.zero_outs]

        def run(self, concat_in, concat_zeros):
            out_arrs = self.sharded(*concat_in, *concat_zeros)
            return [
                {nm: np.asarray(out_arrs[i]).reshape(NCORES, *zero_outs[i].shape)[c]
                 for i, nm in enumerate(self.out_names)}
                for c in range(NCORES)
            ]

    return Executor()


def kernel(**inputs):
    ex = _get_runner()
    in_maps = _prep_inputs(**inputs)
    results = ex.run(ex.concat_inputs(in_maps), ex.fresh_zero_outs())
    return _assemble_outputs(results)


# revision 28
# speedup vs baseline: 3349.5155x; 1.0725x over previous
"""Trainium2 Bass kernel for nn_DiffTreeMachine (B=64, L=16, F=64, R=1023).

Data-parallel over batch: 8 NeuronCores x 8 batches each.

Per core, the computation is
  A_w[b]   = sum_l w_w[b,l] * x[b,l]          (4 weighted reductions, w in {car,cdr,cons1,cons2})
  car      = A_0 @ D_l^T
  cdr      = A_1 @ D_r^T
  cons     = A_2 @ E_l^T + A_3 @ E_r^T + outer(root_filler, root_role)
  ent_w[b] = -(sum_l p log(p+1e-12)) / log(L),  max_w[b] = max_l p

Device mapping:
  Stage 1: for each f (64) and r-chunk (8x128): matmul with x tile as the
    *stationary* operand and a sparse (128, 32) weight-block as the moving
    operand: out[(r),(w,b)] = x_slice.T @ wblk.  This lands A^T directly with
    r on partitions (the layout stage 2 needs for its contraction) with no
    separate transpose pass.  fp32 exact.
  Stage 2: out[(f,b2),(t)] = sum_rc Astage[rc].T @ W^T[rc]  accumulated in
    PSUM over 8 r-chunks, in float32r (PE full rate, ~11-bit-mantissa
    operand rounding, rel err ~1.5e-4).  The cons output accumulates two
    chains (E_l, E_r) plus a K=1 rank-1 matmul for the root term in one
    PSUM group.
"""

import math
import os
import sys
import threading

import numpy as np

for _p in ("/opt/trn_rl_repo", "/root/.axon_site/_ro/trn_rl_repo"):
    if os.path.isdir(_p) and _p not in sys.path:
        sys.path.insert(0, _p)

B, L, F, R = 64, 16, 64, 1023
NCORES = 8
BLOC = B // NCORES          # 8 batches per core
RP = 1024                   # R padded
NRC = 8                     # r-chunks of 128 (last has 127 real rows)
W4 = 4                      # four weight sets
ACOLS = W4 * BLOC * F       # astage cols: w*512 + b*64 + f

_runner_lock = threading.Lock()
_runner = None

# fp16 compute path: x / weight-block / W^T / root vectors stored and fed to
# the PE in fp16 (halves the dominant DMA traffic); PSUM accumulation stays
# fp32.  Measured end-to-end rel err ~5e-4 vs 1.4e-4 for the f32r path.
USE_FP16 = os.environ.get("KERNEL_FP16", "1") == "1"


def _build_program():
    import concourse.tile as tile
    import concourse.mybir as mybir
    from concourse import bacc

    f32 = mybir.dt.float32
    f32r = mybir.dt.float32r
    f16 = mybir.dt.float16
    xdt = f16 if USE_FP16 else f32      # x, wblk (stage-1 operands)
    wdt = f16 if USE_FP16 else f32r     # W^T, astage, rfrr (stage-2 operands)

    nc = bacc.Bacc("TRN2", target_bir_lowering=False, debug=False,
                   num_devices=NCORES)

    xs = nc.declare_dram_parameter("xs", [BLOC * L, F * R], xdt, isOutput=False)
    wblk = nc.declare_dram_parameter("wblk", [BLOC * L, 32], xdt, isOutput=False)
    # W^T padded on both dims: rows r->1024 (zero), cols t->1024 (zero)
    wts = nc.declare_dram_parameter("wts", [W4, RP, RP], wdt, isOutput=False)
    rfrr = nc.declare_dram_parameter("rfrr", [1, 4 * 128 + RP], wdt, isOutput=False)
    pw = nc.declare_dram_parameter("pw", [32, L], f32, isOutput=False)

# row layout: partition p = f*2 + b2 (b2 = batch index within the pair)
    out_car = nc.declare_dram_parameter("out_car", [4, F * 2, R], f32, isOutput=True)
    out_cdr = nc.declare_dram_parameter("out_cdr", [4, F * 2, R], f32, isOutput=True)
    out_cons = nc.declare_dram_parameter("out_cons", [4, F * 2, R], f32, isOutput=True)
    out_stats = nc.declare_dram_parameter("out_stats", [32, 2], f32, isOutput=True)

    outs = {"car": out_car, "cdr": out_cdr, "cons": out_cons}

    with tile.TileContext(nc) as tc:
        with (
            tc.tile_pool(name="const", bufs=1) as const,
            tc.tile_pool(name="astage", bufs=1) as apool,
            tc.tile_pool(name="xp", bufs=6) as xpool,
            tc.tile_pool(name="wtp", bufs=1) as wtpool,
            tc.tile_pool(name="obp", bufs=2) as obpool,
            tc.tile_pool(name="ps1", bufs=1, space="PSUM") as ps1,
            tc.tile_pool(name="ps2", bufs=3, space="PSUM") as ps2,
        ):
            wblk_sb = const.tile([BLOC * L, 32], xdt)
            nc.sync.dma_start(wblk_sb[:], wblk[:])
            # ---- A^T staging: 8 r-chunk tiles, cols = w*512 + b*64 + f ----
            astage = []
            for rc in range(NRC):
                a_t = apool.tile([128, ACOLS], wdt, tag=f"a{rc}", name=f"astage{rc}")
                astage.append(a_t)
            # r = 1023 row (last row of last chunk) is never written by
            # stage 1; zero it so stage 2 reads 0 * 0 there.  DVE partition
            # base must be 32-aligned, so clear the whole last group (stage-1
            # copies then overwrite rows 96..126).
            _last = astage[NRC - 1][96:128, :]
            # (memset has no f32r lowering; bitcast through f32 there)
            nc.vector.memset(_last if USE_FP16 else _last.bitcast(f32), 0.0)

            # ---- stage 1 ----
            # f-blocks of 16; r-chunk halves of 4 so stage 1 uses only 4
            # PSUM banks.  x tiles carry 4 f-slices per DMA.
            for fb in range(4):
                x_ts = []
                for x4 in range(4):
                    f0 = fb * 16 + x4 * 4
                    x_t = xpool.tile([128, 4 * R], xdt, tag="x",
                                     name=f"x{f0}")
                    nc.sync.dma_start(x_t[:], xs[:, f0 * R:(f0 + 4) * R])
                    x_ts.append(x_t)
                for rch in range(2):
                    pst = []
                    for rc2 in range(4):
                        p_t = ps1.tile([128, 512], f32, tag=f"p{rc2}",
                                       name=f"ps1_{rc2}")
                        pst.append(p_t)
                    for f16 in range(16):
                        xsl = x_ts[f16 // 4]
                        xoff = (f16 % 4) * R
                        for rc2 in range(4):
                            rc = rch * 4 + rc2
                            nr = 128 if rc < 7 else 127
                            nc.tensor.matmul(
                                pst[rc2][0:nr, f16 * 32:(f16 + 1) * 32],
                                xsl[:, xoff + rc * 128: xoff + rc * 128 + nr],
                                wblk_sb[:],
                                start=True, stop=True,
                            )
                    for rc2 in range(4):
                        rc = rch * 4 + rc2
                        nr = 128 if rc < 7 else 127
                        # psum cols are f16*32 + w*8 + b; astage cols are
                        # w*512 + b*64 + f.  One strided copy per chunk.
                        src = pst[rc2].rearrange("p (f16 w b) -> p w b f16",
                                                 w=W4, b=BLOC)
                        dst = astage[rc].rearrange("p (w b f) -> p w b f",
                                                   b=BLOC, f=F)
                        nc.vector.tensor_copy(
                            dst[0:nr, :, :, fb * 16:(fb + 1) * 16],
                            src[0:nr, :, :, :],
                        )

            rfrr_sb = const.tile([1, 4 * 128 + RP], wdt)
            nc.sync.dma_start(rfrr_sb[:], rfrr[:])
            pw_sb = const.tile([32, L], f32)
            nc.sync.dma_start(pw_sb[:], pw[:])

            # ---- stats: entropy + max of the four weight matrices ----
            eps = const.tile([32, 1], f32)
            nc.vector.memset(eps[:], 1e-12)
            lnp = const.tile([32, L], f32)
            nc.scalar.activation(lnp[:], pw_sb[:],
                                 mybir.ActivationFunctionType.Ln, bias=eps[:])
            plnp = const.tile([32, L], f32)
            nc.vector.tensor_mul(plnp[:], pw_sb[:], lnp[:])
            st = const.tile([32, 2], f32)
            nc.vector.reduce_sum(st[:, 0:1], plnp[:], axis=mybir.AxisListType.X)
            nc.vector.tensor_scalar_mul(st[:, 0:1], st[:, 0:1], -1.0 / math.log(L))
            nc.vector.reduce_max(st[:, 1:2], pw_sb[:], axis=mybir.AxisListType.X)
            nc.sync.dma_start(out_stats[:], st[:])

            # ---- W^T loads: one DMA per matrix, all resident ----
            # wts[w] rows are rc*128 + r; land as (r partitions, rc*1024 + t)
            wt_sb = []
            for w in range(W4):
                wt_t = wtpool.tile([128, NRC * RP], wdt, tag=f"wt{w}",
                                   name=f"wt{w}")
                wv = wt_t[:].rearrange("p (rc t) -> p rc t", rc=NRC)
                sv = wts[w].rearrange("(rc r) t -> r rc t", r=128)
                nc.sync.dma_start(wv[:, 0:4, :], sv[:, 0:4, :])
                nc.sync.dma_start(wv[:, 4:8, :], sv[:, 4:8, :])
                wt_sb.append(wt_t)

            # ---- stage 2 ----
            # section -> list of w indices to accumulate
            sections = [("car", [0]), ("cdr", [1]), ("cons", [2, 3])]
            for sec, chain in sections:
                od = outs[sec].rearrange("bp p t -> p bp t")
                ob = obpool.tile([128, 4 * RP], f32, tag="ob",
                                 name=f"ob_{sec}")
                n_mm = len(chain) * NRC + (1 if sec == "cons" else 0)
                for bp in range(4):
                    for th in range(2):
                        t0 = th * 512
                        acc = ps2.tile([128, 512], f32, tag="acc",
                                       name=f"acc_{sec}_{bp}_{th}")
                        mm = 0
                        for w in chain:
                            for rc in range(NRC):
                                lhsT = astage[rc][:, w * 512 + bp * 128:
                                                  w * 512 + (bp + 1) * 128]
                                nc.tensor.matmul(
                                    acc[:],
                                    lhsT,
                                    wt_sb[w][:, rc * RP + t0: rc * RP + t0 + 512],
                                    start=(mm == 0), stop=(mm == n_mm - 1),
                                )
                                mm += 1
                        if sec == "cons":
                            nc.tensor.matmul(
                                acc[:],
                                rfrr_sb[0:1, bp * 128:(bp + 1) * 128],
                                rfrr_sb[0:1, 512 + t0: 512 + t0 + 512],
                                start=False, stop=True,
                            )
                        nc.vector.tensor_copy(
                            ob[:, bp * RP + t0: bp * RP + t0 + 512], acc[:])
                    nc.sync.dma_start(
                        od[:, bp, :], ob[:, bp * RP: bp * RP + R])

    nc.compile()
    return nc


def _prep_inputs(x, car_w, cdr_w, cons1_w, cons2_w, root_filler,
                 D_l, D_r, E_l, E_r, root_role):
    """Build the per-core input maps (host-side shard + repack)."""
    cdt = np.float16 if USE_FP16 else np.float32
    x = np.ascontiguousarray(x, dtype=cdt)
    warrs = [np.asarray(a, dtype=np.float32)
             for a in (car_w, cdr_w, cons1_w, cons2_w)]
    root_filler = np.asarray(root_filler, dtype=cdt)
    root_role = np.asarray(root_role, dtype=cdt)

    wts = np.zeros((W4, RP, RP), dtype=cdt)
    for w, Wm in enumerate((D_l, D_r, E_l, E_r)):
        wts[w, :R, :R] = np.asarray(Wm, dtype=np.float32).T

    in_maps = []
    for c in range(NCORES):
        b0 = c * BLOC
        xs = x[b0:b0 + BLOC].reshape(BLOC * L, F * R)

        wblk = np.zeros((BLOC * L, 32), dtype=cdt)
        pw = np.zeros((32, L), dtype=np.float32)
        for w in range(W4):
            for b in range(BLOC):
                wblk[b * L:(b + 1) * L, w * 8 + b] = warrs[w][b0 + b]
                pw[w * 8 + b] = warrs[w][b0 + b]

        rfrr = np.zeros((1, 4 * 128 + RP), dtype=cdt)
        # out partition p = b2*64 + f within a batch pair
        rfrr[0, :512] = root_filler[b0:b0 + BLOC].reshape(512)
        rfrr[0, 512:512 + R] = root_role

        in_maps.append({"xs": xs, "wblk": wblk, "wts": wts,
                        "rfrr": rfrr, "pw": pw})
    return in_maps


def _unshuffle(arr):
    # (4, 2*F, R) with row = b2*64 + f  ->  (8, F, R), b = bp*2+b2
    return arr.reshape(BLOC, F, R)


def _assemble_outputs(results):
    car = np.concatenate(
        [_unshuffle(results[c]["out_car"]) for c in range(NCORES)], axis=0)
    cdr = np.concatenate(
        [_unshuffle(results[c]["out_cdr"]) for c in range(NCORES)], axis=0)
    cons = np.concatenate(
        [_unshuffle(results[c]["out_cons"]) for c in range(NCORES)], axis=0)
    ents = []
    maxes = []
    for w in range(W4):
        e = np.concatenate(
            [results[c]["out_stats"][w * 8:(w + 1) * 8, 0] for c in range(NCORES)])
        m = np.concatenate(
            [results[c]["out_stats"][w * 8:(w + 1) * 8, 1] for c in range(NCORES)])
        ents.append(e.astype(np.float32))
        maxes.append(m.astype(np.float32))
    return (car, cdr, cons) + tuple(ents) + tuple(maxes)


def _get_runner():
    global _runner
    with _runner_lock:
        if _runner is None:
            nc = _build_program()
            _runner = _make_executor(nc)
    return _runner


def _make_executor(nc):
    """Persistent jitted SPMD executor (adapted from bass2jax.run_bass_via_pjrt,
    hoisting the jit so repeated calls don't recompile)."""
    import jax
    import jax.numpy as jnp
    from jax.sharding import Mesh, PartitionSpec
    from jax.experimental.shard_map import shard_map
    import concourse.mybir as mybir
    from concourse import bass2jax

    bass2jax.install_neuronx_cc_hook()

    partition_name = (nc.partition_id_tensor.name
                      if nc.partition_id_tensor else None)
    in_names, out_names, out_avals, zero_outs = [], [], [], []
    for alloc in nc.m.functions[0].allocations:
        if not isinstance(alloc, mybir.MemoryLocationSet):
            continue
        name = alloc.memorylocations[0].name
        if alloc.kind == "ExternalInput":
            if name != partition_name:
                in_names.append(name)
        elif alloc.kind == "ExternalOutput":
            shape = tuple(alloc.tensor_shape)
            dtype = mybir.dt.np(alloc.dtype)
            out_names.append(name)
            out_avals.append(jax.core.ShapedArray(shape, dtype))
            zero_outs.append(np.zeros(shape, dtype))
    n_params = len(in_names)
    n_outs = len(out_avals)
    all_in_names = list(in_names) + list(out_names)
    if partition_name is not None:
        all_in_names.append(partition_name)

    donate = tuple(range(n_params, n_params + n_outs))

    def _body(*args):
        operands = list(args)
        if partition_name is not None:
            operands.append(bass2jax.partition_id_tensor())
        return tuple(bass2jax._bass_exec_p.bind(
            *operands,
            out_avals=tuple(out_avals),
            in_names=tuple(all_in_names),
            out_names=tuple(out_names),
            lowering_input_output_aliases=(),
            sim_require_finite=True,
            sim_require_nnan=True,
            nc=nc,
        ))

    devices = jax.devices()[:NCORES]
    mesh = Mesh(np.asarray(devices), ("core",))
    sharded = jax.jit(
        shard_map(_body, mesh=mesh,
                  in_specs=(PartitionSpec("core"),) * (n_params + n_outs),
                  out_specs=(PartitionSpec("core"),) * n_outs,
                  check_rep=False),
        donate_argnums=donate, keep_unused=True)

    class Executor:
        def __init__(self):
            self.in_names = in_names
            self.out_names = out_names
            self.zero_outs = zero_outs
            self.mesh = mesh
            self.sharded = sharded
            self.n_params = n_params
            self.body = _body
            self.n_outs = n_outs

        def make_chained(self, n):
            """jit that runs the kernel n times back-to-back on device,
            feeding iteration i's outputs as iteration i+1's output buffers.
            Used for timing (amortizes per-call dispatch overhead)."""
            def body_n(*args):
                ins = args[:n_params]
                cur = args[n_params:]
                for _ in range(n):
                    cur = _body(*ins, *cur)
                return cur
            return jax.jit(
                shard_map(body_n, mesh=mesh,
                          in_specs=(PartitionSpec("core"),) * (n_params + n_outs),
                          out_specs=(PartitionSpec("core"),) * n_outs,
                          check_rep=False),
                donate_argnums=donate, keep_unused=True)

        def concat_inputs(self, in_maps):
            return [np.concatenate([np.asarray(in_maps[c][nm])
                                    for c in range(NCORES)], axis=0)
                    for nm in self.in_names]

        def fresh_zero_outs(self):
            return [np.zeros((NCORES * z.shape[0], *z.shape[1:]), z.dtype)
                    for z in self.zero_outs]

        def run(self, concat_in, concat_zeros):
            out_arrs = self.sharded(*concat_in, *concat_zeros)
            return [
                {nm: np.asarray(out_arrs[i]).reshape(NCORES, *zero_outs[i].shape)[c]
                 for i, nm in enumerate(self.out_names)}
                for c in range(NCORES)
            ]

    return Executor()


def kernel(**inputs):
    ex = _get_runner()
    in_maps = _prep_inputs(**inputs)
    results = ex.run(ex.concat_inputs(in_maps), ex.fresh_zero_outs())
    return _assemble_outputs(results)


# revision 30
# speedup vs baseline: 3361.7615x; 1.0037x over previous
"""Trainium2 Bass kernel for nn_DiffTreeMachine (B=64, L=16, F=64, R=1023).

Data-parallel over batch: 8 NeuronCores x 8 batches each.

Per core, the computation is
  A_w[b]   = sum_l w_w[b,l] * x[b,l]          (4 weighted reductions, w in {car,cdr,cons1,cons2})
  car      = A_0 @ D_l^T
  cdr      = A_1 @ D_r^T
  cons     = A_2 @ E_l^T + A_3 @ E_r^T + outer(root_filler, root_role)
  ent_w[b] = -(sum_l p log(p+1e-12)) / log(L),  max_w[b] = max_l p

Device mapping:
  Stage 1: for each f (64) and r-chunk (8x128): matmul with x tile as the
    *stationary* operand and a sparse (128, 32) weight-block as the moving
    operand: out[(r),(w,b)] = x_slice.T @ wblk.  This lands A^T directly with
    r on partitions (the layout stage 2 needs for its contraction) with no
    separate transpose pass.  fp32 exact.
  Stage 2: out[(b2,f),(t)] = sum_rc Astage[rc].T @ W^T[rc]  accumulated in
    PSUM over 8 r-chunks.  The cons output accumulates two chains (E_l,
    E_r) plus a K=1 rank-1 matmul for the root term in one PSUM group.

Default compute dtype is fp16 for PE operands (fp32 PSUM accumulation),
halving the dominant DMA traffic; KERNEL_FP16=0 selects an fp32(stage-1)/
float32r(stage-2) path (rel err 1.4e-4 vs 3.7e-4, ~1.3x slower; note the
f32r path predates the merged-DMA tile shapes and needs smaller pools).

Cost-model timeline (per core): ~115us -- x DMA 47us then PE-bound stage-2
57us; DMA roofline for the fp16 traffic (31.5MB/core @ 360GB/s) is 87us.
"""

import math
import os
import sys
import threading

import numpy as np

for _p in ("/opt/trn_rl_repo", "/root/.axon_site/_ro/trn_rl_repo"):
    if os.path.isdir(_p) and _p not in sys.path:
        sys.path.insert(0, _p)

B, L, F, R = 64, 16, 64, 1023
NCORES = 8
BLOC = B // NCORES          # 8 batches per core
RP = 1024                   # R padded
NRC = 8                     # r-chunks of 128 (last has 127 real rows)
W4 = 4                      # four weight sets
ACOLS = W4 * BLOC * F       # astage cols: w*512 + b*64 + f

_runner_lock = threading.Lock()
_runner = None

# fp16 compute path: x / weight-block / W^T / root vectors stored and fed to
# the PE in fp16 (halves the dominant DMA traffic); PSUM accumulation stays
# fp32.  Measured end-to-end rel err ~5e-4 vs 1.4e-4 for the f32r path.
USE_FP16 = os.environ.get("KERNEL_FP16", "1") == "1"


def _build_program():
    import concourse.tile as tile
    import concourse.mybir as mybir
    from concourse import bacc

    f32 = mybir.dt.float32
    f32r = mybir.dt.float32r
    f16 = mybir.dt.float16
    xdt = f16 if USE_FP16 else f32      # x, wblk (stage-1 operands)
    wdt = f16 if USE_FP16 else f32r     # W^T, astage, rfrr (stage-2 operands)

    nc = bacc.Bacc("TRN2", target_bir_lowering=False, debug=False,
                   num_devices=NCORES)

    xs = nc.declare_dram_parameter("xs", [BLOC * L, F * R], xdt, isOutput=False)
    wblk = nc.declare_dram_parameter("wblk", [BLOC * L, 32], xdt, isOutput=False)
    # W^T padded on both dims: rows r->1024 (zero), cols t->1024 (zero)
    wts = nc.declare_dram_parameter("wts", [W4, RP, RP], wdt, isOutput=False)
    rfrr = nc.declare_dram_parameter("rfrr", [1, 4 * 128 + RP], wdt, isOutput=False)
    pw = nc.declare_dram_parameter("pw", [32, L], f32, isOutput=False)

# row layout: partition p = f*2 + b2 (b2 = batch index within the pair)
    out_car = nc.declare_dram_parameter("out_car", [4, F * 2, R], f32, isOutput=True)
    out_cdr = nc.declare_dram_parameter("out_cdr", [4, F * 2, R], f32, isOutput=True)
    out_cons = nc.declare_dram_parameter("out_cons", [4, F * 2, R], f32, isOutput=True)
    out_stats = nc.declare_dram_parameter("out_stats", [32, 2], f32, isOutput=True)

    outs = {"car": out_car, "cdr": out_cdr, "cons": out_cons}

    with tile.TileContext(nc) as tc:
        with (
            tc.tile_pool(name="const", bufs=1) as const,
            tc.tile_pool(name="astage", bufs=1) as apool,
            tc.tile_pool(name="xp", bufs=6) as xpool,
            tc.tile_pool(name="wtp", bufs=1) as wtpool,
            tc.tile_pool(name="obp", bufs=2) as obpool,
            tc.tile_pool(name="ps1", bufs=1, space="PSUM") as ps1,
            tc.tile_pool(name="ps2", bufs=3, space="PSUM") as ps2,
        ):
            wblk_sb = const.tile([BLOC * L, 32], xdt)
            nc.scalar.dma_start(wblk_sb[:], wblk[:])
            # ---- A^T staging: 8 r-chunk tiles, cols = w*512 + b*64 + f ----
            astage = []
            for rc in range(NRC):
                a_t = apool.tile([128, ACOLS], wdt, tag=f"a{rc}", name=f"astage{rc}")
                astage.append(a_t)
            # r = 1023 row (last row of last chunk) is never written by
            # stage 1; zero it so stage 2 reads 0 * 0 there.  DVE partition
            # base must be 32-aligned, so clear the whole last group (stage-1
            # copies then overwrite rows 96..126).
            _last = astage[NRC - 1][96:128, :]
            # (memset has no f32r lowering; bitcast through f32 there)
            nc.vector.memset(_last if USE_FP16 else _last.bitcast(f32), 0.0)

            # ---- stage 1 ----
            # f-blocks of 16; r-chunk halves of 4 so stage 1 uses only 4
            # PSUM banks.  x tiles carry 4 f-slices per DMA.
            for fb in range(4):
                x_ts = []
                for x4 in range(4):
                    f0 = fb * 16 + x4 * 4
                    x_t = xpool.tile([128, 4 * R], xdt, tag="x",
                                     name=f"x{f0}")
                    nc.sync.dma_start(x_t[:], xs[:, f0 * R:(f0 + 4) * R])
                    x_ts.append(x_t)
                for rch in range(2):
                    pst = []
                    for rc2 in range(4):
                        p_t = ps1.tile([128, 512], f32, tag=f"p{rc2}",
                                       name=f"ps1_{rc2}")
                        pst.append(p_t)
                    for f16 in range(16):
                        xsl = x_ts[f16 // 4]
                        xoff = (f16 % 4) * R
                        for rc2 in range(4):
                            rc = rch * 4 + rc2
                            nr = 128 if rc < 7 else 127
                            nc.tensor.matmul(
                                pst[rc2][0:nr, f16 * 32:(f16 + 1) * 32],
                                xsl[:, xoff + rc * 128: xoff + rc * 128 + nr],
                                wblk_sb[:],
                                start=True, stop=True,
                            )
                    for rc2 in range(4):
                        rc = rch * 4 + rc2
                        nr = 128 if rc < 7 else 127
                        # psum cols are f16*32 + w*8 + b; astage cols are
                        # w*512 + b*64 + f.  One strided copy per chunk.
                        src = pst[rc2].rearrange("p (f16 w b) -> p w b f16",
                                                 w=W4, b=BLOC)
                        dst = astage[rc].rearrange("p (w b f) -> p w b f",
                                                   b=BLOC, f=F)
                        nc.vector.tensor_copy(
                            dst[0:nr, :, :, fb * 16:(fb + 1) * 16],
                            src[0:nr, :, :, :],
                        )

            rfrr_sb = const.tile([1, 4 * 128 + RP], wdt)
            nc.sync.dma_start(rfrr_sb[:], rfrr[:])
            pw_sb = const.tile([32, L], f32)
            nc.sync.dma_start(pw_sb[:], pw[:])

            # ---- stats: entropy + max of the four weight matrices ----
            eps = const.tile([32, 1], f32)
            nc.vector.memset(eps[:], 1e-12)
            lnp = const.tile([32, L], f32)
            nc.scalar.activation(lnp[:], pw_sb[:],
                                 mybir.ActivationFunctionType.Ln, bias=eps[:])
            plnp = const.tile([32, L], f32)
            nc.vector.tensor_mul(plnp[:], pw_sb[:], lnp[:])
            st = const.tile([32, 2], f32)
            nc.vector.reduce_sum(st[:, 0:1], plnp[:], axis=mybir.AxisListType.X)
            nc.vector.tensor_scalar_mul(st[:, 0:1], st[:, 0:1], -1.0 / math.log(L))
            nc.vector.reduce_max(st[:, 1:2], pw_sb[:], axis=mybir.AxisListType.X)
            nc.sync.dma_start(out_stats[:], st[:])

            # ---- W^T loads: one DMA per matrix, all resident ----
            # wts[w] rows are rc*128 + r; land as (r partitions, rc*1024 + t)
            wt_sb = []
            for w in range(W4):
                wt_t = wtpool.tile([128, NRC * RP], wdt, tag=f"wt{w}",
                                   name=f"wt{w}")
                wv = wt_t[:].rearrange("p (rc t) -> p rc t", rc=NRC)
                sv = wts[w].rearrange("(rc r) t -> r rc t", r=128)
                if w == 0:
                    for q in range(4):
                        nc.sync.dma_start(wv[:, 2 * q:2 * q + 2, :],
                                          sv[:, 2 * q:2 * q + 2, :])
                else:
                    nc.sync.dma_start(wv[:, 0:4, :], sv[:, 0:4, :])
                    nc.sync.dma_start(wv[:, 4:8, :], sv[:, 4:8, :])
                wt_sb.append(wt_t)

            # ---- stage 2 ----
            # section -> list of w indices to accumulate
            sections = [("car", [0]), ("cdr", [1]), ("cons", [2, 3])]
            for sec, chain in sections:
                od = outs[sec].rearrange("bp p t -> p bp t")
                ob = obpool.tile([128, 4 * RP], f32, tag="ob",
                                 name=f"ob_{sec}")
                n_mm = len(chain) * NRC + (1 if sec == "cons" else 0)
                for bp in range(4):
                    for th in range(2):
                        t0 = th * 512
                        acc = ps2.tile([128, 512], f32, tag="acc",
                                       name=f"acc_{sec}_{bp}_{th}")
                        mm = 0
                        for w in chain:
                            for rc in range(NRC):
                                lhsT = astage[rc][:, w * 512 + bp * 128:
                                                  w * 512 + (bp + 1) * 128]
                                nc.tensor.matmul(
                                    acc[:],
                                    lhsT,
                                    wt_sb[w][:, rc * RP + t0: rc * RP + t0 + 512],
                                    start=(mm == 0), stop=(mm == n_mm - 1),
                                )
                                mm += 1
                        if sec == "cons":
                            nc.tensor.matmul(
                                acc[:],
                                rfrr_sb[0:1, bp * 128:(bp + 1) * 128],
                                rfrr_sb[0:1, 512 + t0: 512 + t0 + 512],
                                start=False, stop=True,
                            )
                        nc.vector.tensor_copy(
                            ob[:, bp * RP + t0: bp * RP + t0 + 512], acc[:])
                    nc.sync.dma_start(
                        od[:, bp, :], ob[:, bp * RP: bp * RP + R])

    nc.compile()
    return nc


def _prep_inputs(x, car_w, cdr_w, cons1_w, cons2_w, root_filler,
                 D_l, D_r, E_l, E_r, root_role):
    """Build the per-core input maps (host-side shard + repack)."""
    cdt = np.float16 if USE_FP16 else np.float32
    x = np.ascontiguousarray(x, dtype=cdt)
    warrs = [np.asarray(a, dtype=np.float32)
             for a in (car_w, cdr_w, cons1_w, cons2_w)]
    root_filler = np.asarray(root_filler, dtype=cdt)
    root_role = np.asarray(root_role, dtype=cdt)

    wts = np.zeros((W4, RP, RP), dtype=cdt)
    for w, Wm in enumerate((D_l, D_r, E_l, E_r)):
        wts[w, :R, :R] = np.asarray(Wm, dtype=np.float32).T

    in_maps = []
    for c in range(NCORES):
        b0 = c * BLOC
        xs = x[b0:b0 + BLOC].reshape(BLOC * L, F * R)

        wblk = np.zeros((BLOC * L, 32), dtype=cdt)
        pw = np.zeros((32, L), dtype=np.float32)
        for w in range(W4):
            for b in range(BLOC):
                wblk[b * L:(b + 1) * L, w * 8 + b] = warrs[w][b0 + b]
                pw[w * 8 + b] = warrs[w][b0 + b]

        rfrr = np.zeros((1, 4 * 128 + RP), dtype=cdt)
        # out partition p = b2*64 + f within a batch pair
        rfrr[0, :512] = root_filler[b0:b0 + BLOC].reshape(512)
        rfrr[0, 512:512 + R] = root_role

        in_maps.append({"xs": xs, "wblk": wblk, "wts": wts,
                        "rfrr": rfrr, "pw": pw})
    return in_maps


def _unshuffle(arr):
    # (4, 2*F, R) with row = b2*64 + f  ->  (8, F, R), b = bp*2+b2
    return arr.reshape(BLOC, F, R)


def _assemble_outputs(results):
    car = np.concatenate(
        [_unshuffle(results[c]["out_car"]) for c in range(NCORES)], axis=0)
    cdr = np.concatenate(
        [_unshuffle(results[c]["out_cdr"]) for c in range(NCORES)], axis=0)
    cons = np.concatenate(
        [_unshuffle(results[c]["out_cons"]) for c in range(NCORES)], axis=0)
    ents = []
    maxes = []
    for w in range(W4):
        e = np.concatenate(
            [results[c]["out_stats"][w * 8:(w + 1) * 8, 0] for c in range(NCORES)])
        m = np.concatenate(
            [results[c]["out_stats"][w * 8:(w + 1) * 8, 1] for c in range(NCORES)])
        ents.append(e.astype(np.float32))
        maxes.append(m.astype(np.float32))
    return (car, cdr, cons) + tuple(ents) + tuple(maxes)


def _get_runner():
    global _runner
    with _runner_lock:
        if _runner is None:
            nc = _build_program()
            _runner = _make_executor(nc)
    return _runner


def _make_executor(nc):
    """Persistent jitted SPMD executor (adapted from bass2jax.run_bass_via_pjrt,
    hoisting the jit so repeated calls don't recompile)."""
    import jax
    import jax.numpy as jnp
    from jax.sharding import Mesh, PartitionSpec
    from jax.experimental.shard_map import shard_map
    import concourse.mybir as mybir
    from concourse import bass2jax

    bass2jax.install_neuronx_cc_hook()

    partition_name = (nc.partition_id_tensor.name
                      if nc.partition_id_tensor else None)
    in_names, out_names, out_avals, zero_outs = [], [], [], []
    for alloc in nc.m.functions[0].allocations:
        if not isinstance(alloc, mybir.MemoryLocationSet):
            continue
        name = alloc.memorylocations[0].name
        if alloc.kind == "ExternalInput":
            if name != partition_name:
                in_names.append(name)
        elif alloc.kind == "ExternalOutput":
            shape = tuple(alloc.tensor_shape)
            dtype = mybir.dt.np(alloc.dtype)
            out_names.append(name)
            out_avals.append(jax.core.ShapedArray(shape, dtype))
            zero_outs.append(np.zeros(shape, dtype))
    n_params = len(in_names)
    n_outs = len(out_avals)
    all_in_names = list(in_names) + list(out_names)
    if partition_name is not None:
        all_in_names.append(partition_name)

    donate = tuple(range(n_params, n_params + n_outs))

    def _body(*args):
        operands = list(args)
        if partition_name is not None:
            operands.append(bass2jax.partition_id_tensor())
        return tuple(bass2jax._bass_exec_p.bind(
            *operands,
            out_avals=tuple(out_avals),
            in_names=tuple(all_in_names),
            out_names=tuple(out_names),
            lowering_input_output_aliases=(),
            sim_require_finite=True,
            sim_require_nnan=True,
            nc=nc,
        ))

    devices = jax.devices()[:NCORES]
    mesh = Mesh(np.asarray(devices), ("core",))
    sharded = jax.jit(
        shard_map(_body, mesh=mesh,
                  in_specs=(PartitionSpec("core"),) * (n_params + n_outs),
                  out_specs=(PartitionSpec("core"),) * n_outs,
                  check_rep=False),
        donate_argnums=donate, keep_unused=True)

    class Executor:
        def __init__(self):
            self.in_names = in_names
            self.out_names = out_names
            self.zero_outs = zero_outs
            self.mesh = mesh
            self.sharded = sharded
            self.n_params = n_params
            self.body = _body
            self.n_outs = n_outs

        def make_chained(self, n):
            """jit that runs the kernel n times back-to-back on device,
            feeding iteration i's outputs as iteration i+1's output buffers.
            Used for timing (amortizes per-call dispatch overhead)."""
            def body_n(*args):
                ins = args[:n_params]
                cur = args[n_params:]
                for _ in range(n):
                    cur = _body(*ins, *cur)
                return cur
            return jax.jit(
                shard_map(body_n, mesh=mesh,
                          in_specs=(PartitionSpec("core"),) * (n_params + n_outs),
                          out_specs=(PartitionSpec("core"),) * n_outs,
                          check_rep=False),
                donate_argnums=donate, keep_unused=True)

        def concat_inputs(self, in_maps):
            return [np.concatenate([np.asarray(in_maps[c][nm])
                                    for c in range(NCORES)], axis=0)
                    for nm in self.in_names]

        def fresh_zero_outs(self):
            return [np.zeros((NCORES * z.shape[0], *z.shape[1:]), z.dtype)
                    for z in self.zero_outs]

        def run(self, concat_in, concat_zeros):
            out_arrs = self.sharded(*concat_in, *concat_zeros)
            return [
                {nm: np.asarray(out_arrs[i]).reshape(NCORES, *zero_outs[i].shape)[c]
                 for i, nm in enumerate(self.out_names)}
                for c in range(NCORES)
            ]

    return Executor()


def kernel(**inputs):
    ex = _get_runner()
    in_maps = _prep_inputs(**inputs)
    results = ex.run(ex.concat_inputs(in_maps), ex.fresh_zero_outs())
    return _assemble_outputs(results)


# revision 31
# speedup vs baseline: 3384.7944x; 1.0069x over previous
"""Trainium2 Bass kernel for nn_DiffTreeMachine (B=64, L=16, F=64, R=1023).

Data-parallel over batch: 8 NeuronCores x 8 batches each.

Per core, the computation is
  A_w[b]   = sum_l w_w[b,l] * x[b,l]          (4 weighted reductions, w in {car,cdr,cons1,cons2})
  car      = A_0 @ D_l^T
  cdr      = A_1 @ D_r^T
  cons     = A_2 @ E_l^T + A_3 @ E_r^T + outer(root_filler, root_role)
  ent_w[b] = -(sum_l p log(p+1e-12)) / log(L),  max_w[b] = max_l p

Device mapping:
  Stage 1: for each f (64) and r-chunk (8x128): matmul with x tile as the
    *stationary* operand and a sparse (128, 32) weight-block as the moving
    operand: out[(r),(w,b)] = x_slice.T @ wblk.  This lands A^T directly with
    r on partitions (the layout stage 2 needs for its contraction) with no
    separate transpose pass.  fp32 exact.
  Stage 2: out[(b2,f),(t)] = sum_rc Astage[rc].T @ W^T[rc]  accumulated in
    PSUM over 8 r-chunks.  The cons output accumulates two chains (E_l,
    E_r) plus a K=1 rank-1 matmul for the root term in one PSUM group.

Default compute dtype is fp16 for PE operands (fp32 PSUM accumulation),
halving the dominant DMA traffic; KERNEL_FP16=0 selects an fp32(stage-1)/
float32r(stage-2) path (rel err 1.4e-4 vs 3.7e-4, ~1.3x slower; note the
f32r path predates the merged-DMA tile shapes and needs smaller pools).

Cost-model timeline (per core): ~115us -- x DMA 47us then PE-bound stage-2
57us; DMA roofline for the fp16 traffic (31.5MB/core @ 360GB/s) is 87us.
"""

import math
import os
import sys
import threading

import numpy as np

for _p in ("/opt/trn_rl_repo", "/root/.axon_site/_ro/trn_rl_repo"):
    if os.path.isdir(_p) and _p not in sys.path:
        sys.path.insert(0, _p)

B, L, F, R = 64, 16, 64, 1023
NCORES = 8
BLOC = B // NCORES          # 8 batches per core
RP = 1024                   # R padded
NRC = 8                     # r-chunks of 128 (last has 127 real rows)
W4 = 4                      # four weight sets
ACOLS = W4 * BLOC * F       # astage cols: w*512 + b*64 + f

_runner_lock = threading.Lock()
_runner = None

# fp16 compute path: x / weight-block / W^T / root vectors stored and fed to
# the PE in fp16 (halves the dominant DMA traffic); PSUM accumulation stays
# fp32.  Measured end-to-end rel err ~5e-4 vs 1.4e-4 for the f32r path.
USE_FP16 = os.environ.get("KERNEL_FP16", "1") == "1"


def _build_program():
    import concourse.tile as tile
    import concourse.mybir as mybir
    from concourse import bacc

    f32 = mybir.dt.float32
    f32r = mybir.dt.float32r
    f16 = mybir.dt.float16
    xdt = f16 if USE_FP16 else f32      # x, wblk (stage-1 operands)
    wdt = f16 if USE_FP16 else f32r     # W^T, astage, rfrr (stage-2 operands)

    nc = bacc.Bacc("TRN2", target_bir_lowering=False, debug=False,
                   num_devices=NCORES)

    xs = nc.declare_dram_parameter("xs", [BLOC * L, F * R], xdt, isOutput=False)
    wblk = nc.declare_dram_parameter("wblk", [BLOC * L, 32], xdt, isOutput=False)
    # W^T padded on both dims: rows r->1024 (zero), cols t->1024 (zero)
    wts = nc.declare_dram_parameter("wts", [W4, RP, RP], wdt, isOutput=False)
    rfrr = nc.declare_dram_parameter("rfrr", [1, 4 * 128 + RP], wdt, isOutput=False)
    pw = nc.declare_dram_parameter("pw", [32, L], f32, isOutput=False)

# row layout: partition p = f*2 + b2 (b2 = batch index within the pair)
    out_car = nc.declare_dram_parameter("out_car", [4, F * 2, R], f32, isOutput=True)
    out_cdr = nc.declare_dram_parameter("out_cdr", [4, F * 2, R], f32, isOutput=True)
    out_cons = nc.declare_dram_parameter("out_cons", [4, F * 2, R], f32, isOutput=True)
    out_stats = nc.declare_dram_parameter("out_stats", [32, 2], f32, isOutput=True)

    outs = {"car": out_car, "cdr": out_cdr, "cons": out_cons}

    with tile.TileContext(nc) as tc:
        with (
            tc.tile_pool(name="const", bufs=1) as const,
            tc.tile_pool(name="astage", bufs=1) as apool,
            tc.tile_pool(name="xp", bufs=6) as xpool,
            tc.tile_pool(name="wtp", bufs=1) as wtpool,
            tc.tile_pool(name="obp", bufs=2) as obpool,
            tc.tile_pool(name="ps1", bufs=1, space="PSUM") as ps1,
            tc.tile_pool(name="ps2", bufs=3, space="PSUM") as ps2,
        ):
            wblk_sb = const.tile([BLOC * L, 32], xdt)
            nc.scalar.dma_start(wblk_sb[:], wblk[:])
            # ---- A^T staging: 8 r-chunk tiles, cols = w*512 + b*64 + f ----
            astage = []
            for rc in range(NRC):
                a_t = apool.tile([128, ACOLS], wdt, tag=f"a{rc}", name=f"astage{rc}")
                astage.append(a_t)
            # r = 1023 row (last row of last chunk) is never written by
            # stage 1; zero it so stage 2 reads 0 * 0 there.  DVE partition
            # base must be 32-aligned, so clear the whole last group (stage-1
            # copies then overwrite rows 96..126).
            _last = astage[NRC - 1][96:128, :]
            # (memset has no f32r lowering; bitcast through f32 there)
            nc.vector.memset(_last if USE_FP16 else _last.bitcast(f32), 0.0)

            # ---- stage 1 ----
            # f-blocks of 16; r-chunk halves of 4 so stage 1 uses only 4
            # PSUM banks.  x tiles carry 4 f-slices per DMA.
            for fb in range(4):
                x_ts = []
                for x4 in range(4):
                    f0 = fb * 16 + x4 * 4
                    x_t = xpool.tile([128, 4 * R], xdt, tag="x",
                                     name=f"x{f0}")
                    nc.sync.dma_start(x_t[:], xs[:, f0 * R:(f0 + 4) * R])
                    x_ts.append(x_t)
                for rch in range(2):
                    pst = []
                    for rc2 in range(4):
                        p_t = ps1.tile([128, 512], f32, tag=f"p{rc2}",
                                       name=f"ps1_{rc2}")
                        pst.append(p_t)
                    for f16 in range(16):
                        xsl = x_ts[f16 // 4]
                        xoff = (f16 % 4) * R
                        for rc2 in range(4):
                            rc = rch * 4 + rc2
                            nr = 128 if rc < 7 else 127
                            nc.tensor.matmul(
                                pst[rc2][0:nr, f16 * 32:(f16 + 1) * 32],
                                xsl[:, xoff + rc * 128: xoff + rc * 128 + nr],
                                wblk_sb[:],
                                start=True, stop=True,
                            )
                    for rc2 in range(4):
                        rc = rch * 4 + rc2
                        nr = 128 if rc < 7 else 127
                        # psum cols are f16*32 + w*8 + b; astage cols are
                        # w*512 + b*64 + f.  One strided copy per chunk.
                        src = pst[rc2].rearrange("p (f16 w b) -> p w b f16",
                                                 w=W4, b=BLOC)
                        dst = astage[rc].rearrange("p (w b f) -> p w b f",
                                                   b=BLOC, f=F)
                        nc.vector.tensor_copy(
                            dst[0:nr, :, :, fb * 16:(fb + 1) * 16],
                            src[0:nr, :, :, :],
                        )

            rfrr_sb = const.tile([1, 4 * 128 + RP], wdt)
            nc.sync.dma_start(rfrr_sb[:], rfrr[:])
            pw_sb = const.tile([32, L], f32)
            nc.sync.dma_start(pw_sb[:], pw[:])

            # ---- stats: entropy + max of the four weight matrices ----
            eps = const.tile([32, 1], f32)
            nc.vector.memset(eps[:], 1e-12)
            lnp = const.tile([32, L], f32)
            nc.scalar.activation(lnp[:], pw_sb[:],
                                 mybir.ActivationFunctionType.Ln, bias=eps[:])
            plnp = const.tile([32, L], f32)
            nc.vector.tensor_mul(plnp[:], pw_sb[:], lnp[:])
            st = const.tile([32, 2], f32)
            nc.vector.reduce_sum(st[:, 0:1], plnp[:], axis=mybir.AxisListType.X)
            nc.vector.tensor_scalar_mul(st[:, 0:1], st[:, 0:1], -1.0 / math.log(L))
            nc.vector.reduce_max(st[:, 1:2], pw_sb[:], axis=mybir.AxisListType.X)
            nc.sync.dma_start(out_stats[:], st[:])

            # ---- W^T loads: one DMA per matrix, all resident ----
            # wts[w] rows are rc*128 + r; land as (r partitions, rc*1024 + t)
            wt_sb = []
            for w in range(W4):
                wt_t = wtpool.tile([128, NRC * RP], wdt, tag=f"wt{w}",
                                   name=f"wt{w}")
                wv = wt_t[:].rearrange("p (rc t) -> p rc t", rc=NRC)
                sv = wts[w].rearrange("(rc r) t -> r rc t", r=128)
                if w == 0:
                    for q in range(4):
                        nc.sync.dma_start(wv[:, 2 * q:2 * q + 2, :],
                                          sv[:, 2 * q:2 * q + 2, :])
                else:
                    nc.sync.dma_start(wv[:, 0:4, :], sv[:, 0:4, :])
                    nc.sync.dma_start(wv[:, 4:8, :], sv[:, 4:8, :])
                wt_sb.append(wt_t)

            # ---- stage 2 ----
            # section -> list of w indices to accumulate
            sections = [("car", [0]), ("cdr", [1]), ("cons", [2, 3])]
            for sec, chain in sections:
                od = outs[sec].rearrange("bp p t -> p bp t")
                ob = obpool.tile([128, 4 * RP], f32, tag="ob",
                                 name=f"ob_{sec}")
                n_mm = len(chain) * NRC + (1 if sec == "cons" else 0)
                for bp in range(4):
                    for th in range(2):
                        t0 = th * 512
                        acc = ps2.tile([128, 512], f32, tag="acc",
                                       name=f"acc_{sec}_{bp}_{th}")
                        mm = 0
                        for w in chain:
                            for rc in range(NRC):
                                lhsT = astage[rc][:, w * 512 + bp * 128:
                                                  w * 512 + (bp + 1) * 128]
                                nc.tensor.matmul(
                                    acc[:],
                                    lhsT,
                                    wt_sb[w][:, rc * RP + t0: rc * RP + t0 + 512],
                                    start=(mm == 0), stop=(mm == n_mm - 1),
                                )
                                mm += 1
                        if sec == "cons":
                            nc.tensor.matmul(
                                acc[:],
                                rfrr_sb[0:1, bp * 128:(bp + 1) * 128],
                                rfrr_sb[0:1, 512 + t0: 512 + t0 + 512],
                                start=False, stop=True,
                            )
                        nc.vector.tensor_copy(
                            ob[:, bp * RP + t0: bp * RP + t0 + 512], acc[:])
                        nt = 512 if th == 0 else R - 512
                        nc.sync.dma_start(
                            od[:, bp, t0:t0 + nt],
                            ob[:, bp * RP + t0: bp * RP + t0 + nt])

    nc.compile()
    return nc


def _prep_inputs(x, car_w, cdr_w, cons1_w, cons2_w, root_filler,
                 D_l, D_r, E_l, E_r, root_role):
    """Build the per-core input maps (host-side shard + repack)."""
    cdt = np.float16 if USE_FP16 else np.float32
    x = np.ascontiguousarray(x, dtype=cdt)
    warrs = [np.asarray(a, dtype=np.float32)
             for a in (car_w, cdr_w, cons1_w, cons2_w)]
    root_filler = np.asarray(root_filler, dtype=cdt)
    root_role = np.asarray(root_role, dtype=cdt)

    wts = np.zeros((W4, RP, RP), dtype=cdt)
    for w, Wm in enumerate((D_l, D_r, E_l, E_r)):
        wts[w, :R, :R] = np.asarray(Wm, dtype=np.float32).T

    in_maps = []
    for c in range(NCORES):
        b0 = c * BLOC
        xs = x[b0:b0 + BLOC].reshape(BLOC * L, F * R)

        wblk = np.zeros((BLOC * L, 32), dtype=cdt)
        pw = np.zeros((32, L), dtype=np.float32)
        for w in range(W4):
            for b in range(BLOC):
                wblk[b * L:(b + 1) * L, w * 8 + b] = warrs[w][b0 + b]
                pw[w * 8 + b] = warrs[w][b0 + b]

        rfrr = np.zeros((1, 4 * 128 + RP), dtype=cdt)
        # out partition p = b2*64 + f within a batch pair
        rfrr[0, :512] = root_filler[b0:b0 + BLOC].reshape(512)
        rfrr[0, 512:512 + R] = root_role

        in_maps.append({"xs": xs, "wblk": wblk, "wts": wts,
                        "rfrr": rfrr, "pw": pw})
    return in_maps


def _unshuffle(arr):
    # (4, 2*F, R) with row = b2*64 + f  ->  (8, F, R), b = bp*2+b2
    return arr.reshape(BLOC, F, R)


def _assemble_outputs(results):
    car = np.concatenate(
        [_unshuffle(results[c]["out_car"]) for c in range(NCORES)], axis=0)
    cdr = np.concatenate(
        [_unshuffle(results[c]["out_cdr"]) for c in range(NCORES)], axis=0)
    cons = np.concatenate(
        [_unshuffle(results[c]["out_cons"]) for c in range(NCORES)], axis=0)
    ents = []
    maxes = []
    for w in range(W4):
        e = np.concatenate(
            [results[c]["out_stats"][w * 8:(w + 1) * 8, 0] for c in range(NCORES)])
        m = np.concatenate(
            [results[c]["out_stats"][w * 8:(w + 1) * 8, 1] for c in range(NCORES)])
        ents.append(e.astype(np.float32))
        maxes.append(m.astype(np.float32))
    return (car, cdr, cons) + tuple(ents) + tuple(maxes)


def _get_runner():
    global _runner
    with _runner_lock:
        if _runner is None:
            nc = _build_program()
            _runner = _make_executor(nc)
    return _runner


def _make_executor(nc):
    """Persistent jitted SPMD executor (adapted from bass2jax.run_bass_via_pjrt,
    hoisting the jit so repeated calls don't recompile)."""
    import jax
    import jax.numpy as jnp
    from jax.sharding import Mesh, PartitionSpec
    from jax.experimental.shard_map import shard_map
    import concourse.mybir as mybir
    from concourse import bass2jax

    bass2jax.install_neuronx_cc_hook()

    partition_name = (nc.partition_id_tensor.name
                      if nc.partition_id_tensor else None)
    in_names, out_names, out_avals, zero_outs = [], [], [], []
    for alloc in nc.m.functions[0].allocations:
        if not isinstance(alloc, mybir.MemoryLocationSet):
            continue
        name = alloc.memorylocations[0].name
        if alloc.kind == "ExternalInput":
            if name != partition_name:
                in_names.append(name)
        elif alloc.kind == "ExternalOutput":
            shape = tuple(alloc.tensor_shape)
            dtype = mybir.dt.np(alloc.dtype)
            out_names.append(name)
            out_avals.append(jax.core.ShapedArray(shape, dtype))
            zero_outs.append(np.zeros(shape, dtype))
    n_params = len(in_names)
    n_outs = len(out_avals)
    all_in_names = list(in_names) + list(out_names)
    if partition_name is not None:
        all_in_names.append(partition_name)

    donate = tuple(range(n_params, n_params + n_outs))

    def _body(*args):
        operands = list(args)
        if partition_name is not None:
            operands.append(bass2jax.partition_id_tensor())
        return tuple(bass2jax._bass_exec_p.bind(
            *operands,
            out_avals=tuple(out_avals),
            in_names=tuple(all_in_names),
            out_names=tuple(out_names),
            lowering_input_output_aliases=(),
            sim_require_finite=True,
            sim_require_nnan=True,
            nc=nc,
        ))

    devices = jax.devices()[:NCORES]
    mesh = Mesh(np.asarray(devices), ("core",))
    sharded = jax.jit(
        shard_map(_body, mesh=mesh,
                  in_specs=(PartitionSpec("core"),) * (n_params + n_outs),
                  out_specs=(PartitionSpec("core"),) * n_outs,
                  check_rep=False),
        donate_argnums=donate, keep_unused=True)

    class Executor:
        def __init__(self):
            self.in_names = in_names
            self.out_names = out_names
            self.zero_outs = zero_outs
            self.mesh = mesh
            self.sharded = sharded
            self.n_params = n_params
            self.body = _body
            self.n_outs = n_outs

        def make_chained(self, n):
            """jit that runs the kernel n times back-to-back on device,
            feeding iteration i's outputs as iteration i+1's output buffers.
            Used for timing (amortizes per-call dispatch overhead)."""
            def body_n(*args):
                ins = args[:n_params]
                cur = args[n_params:]
                for _ in range(n):
                    cur = _body(*ins, *cur)
                return cur
            return jax.jit(
                shard_map(body_n, mesh=mesh,
                          in_specs=(PartitionSpec("core"),) * (n_params + n_outs),
                          out_specs=(PartitionSpec("core"),) * n_outs,
                          check_rep=False),
                donate_argnums=donate, keep_unused=True)

        def concat_inputs(self, in_maps):
            return [np.concatenate([np.asarray(in_maps[c][nm])
                                    for c in range(NCORES)], axis=0)
                    for nm in self.in_names]

        def fresh_zero_outs(self):
            return [np.zeros((NCORES * z.shape[0], *z.shape[1:]), z.dtype)
                    for z in self.zero_outs]

        def run(self, concat_in, concat_zeros):
            out_arrs = self.sharded(*concat_in, *concat_zeros)
            return [
                {nm: np.asarray(out_arrs[i]).reshape(NCORES, *zero_outs[i].shape)[c]
                 for i, nm in enumerate(self.out_names)}
                for c in range(NCORES)
            ]

    return Executor()


def kernel(**inputs):
    ex = _get_runner()
    in_maps = _prep_inputs(**inputs)
    results = ex.run(ex.concat_inputs(in_maps), ex.fresh_zero_outs())
    return _assemble_outputs(results)
